# revision 16
# baseline (speedup 1.0000x reference)
"""3-layer GAT on 8 TRN2 NeuronCores.

Sharding: nodes partitioned by dst across 8 cores; per-destination edge
batches (<=128 distinct dst, fixed tile budget) aggregated via one-hot
selection matmuls in PSUM; per-edge source rows fetched with dma_gather
from a per-layer feature table [h | asrc]; adst fetched from a core-local
narrow table; softmax done unnormalized (exp / segment-sum via an extra
payload column).

Transfer-optimized: the only per-core H2D payloads are the x shard
(transposed), compact 16-partition gather indices (replicated to 128
partitions on device), int8 slot metadata, i32 scatter rows, and the
(small) weights. The full layer-1 feature table is built on device from
an AllGather of the x shards; layers 2/3 AllGather their own h shards.
A jitted shard_map runner is cached so repeat calls skip retracing, and
donated output buffers are created device-side (no H2D for them).
"""
import numpy as np

from concourse import bass, bacc, mybir, tile

f32 = mybir.dt.float32
bf16 = mybir.dt.bfloat16
i8 = mybir.dt.int8
i16 = mybir.dt.int16
i32 = mybir.dt.int32
Alu = mybir.AluOpType
Act = mybir.ActivationFunctionType
IOA = bass.IndirectOffsetOnAxis

FULL_CFG = dict(
    N=50000, IN=128, HID=64, OUT=64, NH=4, E=800000, R=8,
    T=17, TL=11, TH=6,            # tiles per batch: low-src + high-src
    VSPLIT=32768,                 # int16 index reach for the fat gather
)


def make_cfg(**over):
    cfg = dict(FULL_CFG)
    cfg.update(over)
    N, R = cfg["N"], cfg["R"]
    assert N % R == 0
    cfg["SHARD"] = N // R
    # local rows: shard + >=2 pad rows, multiple of 128
    cfg["LPAD"] = ((cfg["SHARD"] + 2 + 127) // 128) * 128
    cfg["TROWS"] = R * cfg["LPAD"]
    cfg["PADROW"] = cfg["SHARD"]  # local pad row (asrc=-1e30 in every shard)
    cfg["TRASH"] = cfg["LPAD"] - 1
    if cfg["TROWS"] <= cfg["VSPLIT"]:
        cfg["VSPLIT"] = cfg["TROWS"]
        cfg["TL"] = cfg["TL"] + cfg["TH"]
        cfg["TH"] = 0
    else:
        assert cfg["TROWS"] - cfg["VSPLIT"] <= 32768
        # need a pad row in the high range: core r covers
        # [r*LPAD, r*LPAD+SHARD) real + pads; find r with pad row >= VSPLIT
        r = 0
        while r * cfg["LPAD"] + cfg["SHARD"] < cfg["VSPLIT"]:
            r += 1
        cfg["PADROW_H"] = r * cfg["LPAD"] + cfg["SHARD"]
        assert cfg["PADROW_H"] >= cfg["VSPLIT"]
    # fat table row widths (fp32, multiple of 64 elems = 256B)
    cfg["FATW12"] = 320   # h(256) | asrc(4) | pad
    cfg["FATW3"] = 128    # h(64) | asrc(1) | pad
    cfg["NRW"] = 64       # narrow adst table row width
    cfg["IXC"] = (cfg["TL"] + cfg["TH"] + cfg["T"]) * 8  # idx cols per batch
    return cfg


def _renum(n, cfg):
    return (n // cfg["SHARD"]) * cfg["LPAD"] + (n % cfg["SHARD"])


def _to_bf16(a):
    import ml_dtypes
    return np.asarray(a, np.float32).astype(ml_dtypes.bfloat16)


def _wrap_idx(idx_flat):
    """dma_gather int16 index layout (compact): ordinal i at [i%16, i//16]."""
    n = len(idx_flat)
    assert n % 16 == 0
    return np.asarray(idx_flat, np.int16).reshape(n // 16, 16).T


def _pack_core(src_g, dst_l, cfg):
    """Pack one core's edges (global renumbered src, local dst) into batches.

    Returns list of dicts with per-batch arrays.
    """
    T, TL, TH = cfg["T"], cfg["TL"], cfg["TH"]
    VS = cfg["VSPLIT"]
    capL, capH = TL * 128, TH * 128
    order = np.argsort(dst_l, kind="stable")
    src_g = src_g[order]
    dst_l = dst_l[order]
    nodes, starts, counts = np.unique(dst_l, return_index=True, return_counts=True)

    batches = []

    def new_batch():
        return dict(nodes=[], eL=[], eH=[], sL=[], sH=[])

    def flush(b):
        if b is None or not b["nodes"]:
            return
        batches.append(b)

    cur = new_batch()
    for node, st, cnt in zip(nodes, starts, counts):
        s = src_g[st:st + cnt]
        low = s < VS
        nL, nH = int(low.sum()), int(cnt - low.sum())
        if (len(cur["nodes"]) >= 128 or len(cur["eL"]) + nL > capL
                or len(cur["eH"]) + nH > capH):
            flush(cur)
            cur = new_batch()
        slot = len(cur["nodes"])
        cur["nodes"].append(int(node))
        cur["eL"].extend(s[low].tolist())
        cur["sL"].extend([slot] * nL)
        cur["eH"].extend(s[~low].tolist())
        cur["sH"].extend([slot] * nH)
    flush(cur)
    return batches


def _batch_arrays(batches, B, cfg):
    """Build stacked per-batch device arrays for one core.

    Returns:
      idxc  [16, B*IXC] i16 — compact gather indices (low | high | dloc per batch)
      mf8   [128, B*T]  i8  — per-edge slot ids
      lid32 [128, B]    i32 — slot -> local out row (TRASH for unused slots)
    """
    T, TL, TH = cfg["T"], cfg["TL"], cfg["TH"]
    VS = cfg["VSPLIT"]
    IXC = cfg["IXC"]
    PAD_L = cfg["PADROW"]          # low-range pad row (core 0 local == global)
    PAD_H = cfg.get("PADROW_H", 0)
    idxc = np.zeros((16, B * IXC), np.int16)
    mf8 = np.zeros((128, B * T), np.int8)
    lid32 = np.full((128, B), cfg["TRASH"], np.int32)
    for bi in range(B):
        if bi < len(batches):
            b = batches[bi]
            nodes = b["nodes"]
            eL, sL = b["eL"], b["sL"]
            eH, sH = b["eH"], b["sH"]
        else:
            nodes, eL, sL, eH, sH = [], [], [], [], []
        # low gather indices (pad with PAD_L)
        iL = np.full(TL * 128, PAD_L, np.int64)
        iL[:len(eL)] = eL
        iH = np.full(TH * 128, max(PAD_H - VS, 0), np.int64)
        if eH:
            iH[:len(eH)] = np.asarray(eH) - VS
        # per-edge slot array in ordinal order (L block then H block)
        slots = np.zeros(T * 128, np.int64)
        slots[:len(sL)] = sL
        slots[TL * 128:TL * 128 + len(sH)] = sH
        # per-edge local-dst row for the narrow adst gather
        nodes_a = np.asarray(nodes, np.int64) if nodes else np.zeros(0, np.int64)
        dloc = np.full(T * 128, PAD_L, np.int64)
        if len(sL):
            dloc[:len(sL)] = nodes_a[np.asarray(sL)]
        if len(sH):
            dloc[TL * 128:TL * 128 + len(sH)] = nodes_a[np.asarray(sH)]
        c = bi * IXC
        idxc[:, c:c + TL * 8] = _wrap_idx(iL); c += TL * 8
        if TH:
            idxc[:, c:c + TH * 8] = _wrap_idx(iH); c += TH * 8
        idxc[:, c:c + T * 8] = _wrap_idx(dloc)
        mf8[:, bi * T:(bi + 1) * T] = slots.reshape(T, 128).T
        lid32[:len(nodes), bi] = nodes
    return idxc, mf8, lid32


def prep_host(x, edge_index, cfg):
    """All host-side sharding prep. Returns (per_core dicts, B)."""
    N, R, SHARD, LPAD = cfg["N"], cfg["R"], cfg["SHARD"], cfg["LPAD"]
    IN = cfg["IN"]
    src = np.concatenate([np.asarray(edge_index[0]), np.arange(N)]).astype(np.int64)
    dst = np.concatenate([np.asarray(edge_index[1]), np.arange(N)]).astype(np.int64)
    src_g = _renum(src, cfg)

    per_core_batches = []
    for r in range(R):
        m = (dst // SHARD) == r
        per_core_batches.append(_pack_core(src_g[m], dst[m] - r * SHARD, cfg))
    B = max(len(b) for b in per_core_batches)

    per_core = []
    for r in range(R):
        idxc, mf8, lid32 = _batch_arrays(per_core_batches[r], B, cfg)
        xm = np.zeros((IN, LPAD), np.float32)
        xm[:, :SHARD] = np.asarray(x[r * SHARD:(r + 1) * SHARD]).T
        per_core.append(dict(idxc=idxc, mf8=mf8, lid32=lid32,
                             xmine=_to_bf16(xm)))
    return per_core, B


def _aug_w(W, a_s, a_d, nh, hid):
    """[inF, outF+2*nh] = [W.T | As | Ad]."""
    inf = W.shape[1]
    Wr = W.reshape(nh, hid, inf)
    As = np.einsum("hci,hc->ih", Wr, a_s)
    Ad = np.einsum("hci,hc->ih", Wr, a_d)
    return np.concatenate([W.T, As, Ad], axis=1).astype(np.float32)


def build_nc(cfg, B):
    N, R = cfg["N"], cfg["R"]
    LPAD, TROWS, SHARD = cfg["LPAD"], cfg["TROWS"], cfg["SHARD"]
    T, TL, TH = cfg["T"], cfg["TL"], cfg["TH"]
    VS = cfg["VSPLIT"]
    NH, HID, OUT, IN = cfg["NH"], cfg["HID"], cfg["OUT"], cfg["IN"]
    F = NH * HID              # 256
    FATW, FATW3, NRW = cfg["FATW12"], cfg["FATW3"], cfg["NRW"]
    IXC = cfg["IXC"]
    NLT = LPAD // 128

    nc = bacc.Bacc("TRN2", target_bir_lowering=False, debug=False, num_devices=R)

    P = {}
    P["xmine"] = nc.declare_dram_parameter("xmine", [IN, LPAD], bf16, isOutput=False)
    P["w1t"] = nc.declare_dram_parameter("w1t", [IN, F + 2 * NH], f32, isOutput=False)
    P["w2t"] = nc.declare_dram_parameter("w2t", [F, F + 2 * NH], f32, isOutput=False)
    P["w3t"] = nc.declare_dram_parameter("w3t", [F, OUT + 2], f32, isOutput=False)
    P["bvec"] = nc.declare_dram_parameter("bvec", [1, 2 * F + OUT], f32, isOutput=False)
    P["idxc"] = nc.declare_dram_parameter("idxc", [16, B * IXC], i16, isOutput=False)
    P["mf8"] = nc.declare_dram_parameter("mf8", [128, B * T], i8, isOutput=False)
    P["lid32"] = nc.declare_dram_parameter("lid32", [128, B], i32, isOutput=False)
    out_p = nc.declare_dram_parameter("out", [LPAD, OUT], bf16, isOutput=True)

    x0 = nc.dram_tensor("x0", [IN, LPAD], bf16)
    xg = nc.dram_tensor("xg", [R * IN, LPAD], bf16, addr_space="Shared")
    tbl1 = nc.dram_tensor("tbl1", [TROWS, FATW], f32)
    tbl2 = nc.dram_tensor("tbl2", [TROWS, FATW], f32, addr_space="Shared")
    tbl3 = nc.dram_tensor("tbl3", [TROWS, FATW3], f32, addr_space="Shared")
    own_h2 = nc.dram_tensor("own_h2", [LPAD, FATW], f32)
    own_h3 = nc.dram_tensor("own_h3", [LPAD, FATW3], f32)
    adl1 = nc.dram_tensor("adl1", [LPAD, NRW], f32)
    adl2 = nc.dram_tensor("adl2", [LPAD, NRW], f32)
    adl3 = nc.dram_tensor("adl3", [LPAD, NRW], f32)
    own_x1 = nc.dram_tensor("own_x1", [LPAD, F], f32)
    own_x2 = nc.dram_tensor("own_x2", [LPAD, F], f32)

    with tile.TileContext(nc) as tc:
        with tc.tile_pool(name="const", bufs=1) as cpool, \
             tc.tile_pool(name="work", bufs=3) as wpool, \
             tc.tile_pool(name="gath", bufs=2) as gpool, \
             tc.tile_pool(name="psA", bufs=2, space="PSUM") as psA, \
             tc.tile_pool(name="psB", bufs=2, space="PSUM") as psB, \
             tc.tile_pool(name="psC", bufs=2, space="PSUM") as psC:

            def load_const(name, shape, dtype=f32):
                t = cpool.tile(shape, dtype, tag=name)
                nc.sync.dma_start(out=t[:], in_=P[name][:])
                return t

            w1t = load_const("w1t", [IN, F + 2 * NH])
            w2t_lo = cpool.tile([128, F + 2 * NH], f32, tag="w2lo")
            nc.sync.dma_start(out=w2t_lo[:], in_=P["w2t"][0:128, :])
            w2t_hi = cpool.tile([128, F + 2 * NH], f32, tag="w2hi")
            nc.sync.dma_start(out=w2t_hi[:], in_=P["w2t"][128:256, :])
            w3t_lo = cpool.tile([128, OUT + 2], f32, tag="w3lo")
            nc.sync.dma_start(out=w3t_lo[:], in_=P["w3t"][0:128, :])
            w3t_hi = cpool.tile([128, OUT + 2], f32, tag="w3hi")
            nc.sync.dma_start(out=w3t_hi[:], in_=P["w3t"][128:256, :])

            # x shard resident in SBUF (for adl1 build; bf16->f32 cast in DMA)
            xm = cpool.tile([IN, LPAD], f32, tag="xm")
            nc.gpsimd.dma_start(out=xm[:], in_=P["xmine"][:])

            # gather indices: load compact [16, B*IXC] into each 16-partition
            # group (8 replicated loads), resident for all layers
            ix_all = cpool.tile([128, B * IXC], i16, tag="ixall")
            for r8 in range(8):
                nc.sync.dma_start(out=ix_all[r8 * 16:(r8 + 1) * 16, :],
                                  in_=P["idxc"][:])
            mf_all = cpool.tile([128, B * T], i8, tag="mfall")
            nc.sync.dma_start(out=mf_all[:], in_=P["mf8"][:])
            lid_all = cpool.tile([128, B], i32, tag="lidall")
            nc.sync.dma_start(out=lid_all[:], in_=P["lid32"][:])

            # device-built constants: iof (row index ramp), identity, biases
            it_a = cpool.tile([128, 128], i32, tag="ita")
            nc.gpsimd.iota(it_a[:], pattern=[[1, 128]], channel_multiplier=0)
            it_b = cpool.tile([128, 128], i32, tag="itb")
            nc.gpsimd.iota(it_b[:], pattern=[[0, 128]], channel_multiplier=1)
            iof = cpool.tile([128, 128], f32, tag="iof")
            nc.vector.tensor_copy(iof[:], it_a[:])
            pidx = cpool.tile([128, 128], f32, tag="pidx")
            nc.vector.tensor_copy(pidx[:], it_b[:])
            ident = cpool.tile([128, 128], f32, tag="ident")
            nc.vector.tensor_tensor(ident[:], iof[:], pidx[:], Alu.is_equal)

            bv = cpool.tile([1, 2 * F + OUT], f32, tag="bv")
            nc.sync.dma_start(out=bv[:], in_=P["bvec"][:])
            ones1 = cpool.tile([1, 128], f32, tag="ones1")
            nc.vector.memset(ones1[:], 1.0)
            b1 = cpool.tile([128, F], f32, tag="b1")
            b2 = cpool.tile([128, F], f32, tag="b2")
            b3 = cpool.tile([128, OUT], f32, tag="b3")
            for bias_t, off, w in ((b1, 0, F), (b2, F, F), (b3, 2 * F, OUT)):
                psb = psA.tile([128, F + 2 * NH], f32, tag="dens")
                nc.tensor.matmul(psb[:, :w], lhsT=ones1[:], rhs=bv[:, off:off + w],
                                 start=True, stop=True)
                nc.vector.tensor_copy(bias_t[:], psb[:, :w])

            zero = cpool.tile([128, F], f32, tag="zero")
            nc.vector.memset(zero[:], 0.0)
            neg = cpool.tile([128, NH], f32, tag="neg")
            nc.vector.memset(neg[:], -1e30)

            # -------- AllGather x shards -> xg [R*IN, LPAD] ------------------
            # (collectives cannot read IO tensors; stage through x0)
            nc.sync.dma_start(out=x0[:], in_=P["xmine"][:])
            nc.gpsimd.collective_compute(
                "AllGather", Alu.bypass, replica_groups=[list(range(R))],
                ins=[x0[:].opt()], outs=[xg[:].opt()])

            # ---------------- L1 dense: full table1 = [h1|asrc1] -------------
            for rblk in range(R):
                for t in range(NLT):
                    xc = wpool.tile([IN, 128], f32, tag="xc")
                    nc.gpsimd.dma_start(
                        out=xc[:],
                        in_=xg[rblk * IN:(rblk + 1) * IN, t * 128:(t + 1) * 128])
                    ps = psA.tile([128, F + 2 * NH], f32, tag="dens")
                    nc.tensor.matmul(ps[:], lhsT=xc[:], rhs=w1t[:],
                                     start=True, stop=True)
                    hrow = wpool.tile([128, FATW], f32, tag="hrow")
                    if t % 2 == 0:
                        nc.vector.tensor_copy(hrow[:, :F + NH], ps[:, :F + NH])
                    else:
                        nc.scalar.activation(hrow[:, :F + NH], ps[:, :F + NH], Act.Copy)
                    nc.vector.memset(hrow[:, F + NH:], 0.0)
                    row0 = rblk * LPAD + t * 128
                    nc.sync.dma_start(out=tbl1[row0:row0 + 128, :], in_=hrow[:])
            npad = LPAD - SHARD
            nc.sync.dma_start(out=tbl1[SHARD:LPAD, F:F + NH], in_=neg[:npad, :])
            if TH:
                ph = cfg["PADROW_H"]
                nc.sync.dma_start(out=tbl1[ph:ph + npad, F:F + NH], in_=neg[:npad, :])

            # L1 local adst table (from resident x shard)
            for t in range(NLT):
                ps = psB.tile([128, NH], f32, tag="adl")
                nc.tensor.matmul(ps[:], lhsT=xm[:, t * 128:(t + 1) * 128],
                                 rhs=w1t[:, F + NH:F + 2 * NH],
                                 start=True, stop=True)
                ad = wpool.tile([128, NRW], f32, tag="ad")
                nc.vector.tensor_copy(ad[:, 0:NH], ps[:])
                nc.vector.memset(ad[:, NH:], 0.0)
                nc.sync.dma_start(out=adl1[t * 128:(t + 1) * 128, :], in_=ad[:])

            # ---------------- generic agg layer ------------------------------
            def agg_layer(tbl, adl, fatw, nh, c, payw, bias, relu, out_dram, outw,
                          out_dt=f32):
                # payw = nh*c + nh ; outw = nh*c
                for b in range(B):
                    ixb = b * IXC
                    gat = gpool.tile([128, T * fatw], f32, tag="gat")
                    g3 = gat[:].rearrange("p (t q) -> p t q", q=fatw)
                    SP = False  # single_packet overflows DMA packet limits here
                    nc.gpsimd.dma_gather(
                        g3[:, 0:TL, :], tbl[0:VS, :],
                        ix_all[:, ixb:ixb + TL * 8],
                        TL * 128, TL * 128, fatw, single_packet=SP)
                    if TH:
                        nc.gpsimd.dma_gather(
                            g3[:, TL:T, :], tbl[VS:TROWS, :],
                            ix_all[:, ixb + TL * 8:ixb + (TL + TH) * 8],
                            TH * 128, TH * 128, fatw, single_packet=SP)
                    nrg = gpool.tile([128, T * NRW], f32, tag="nrg")
                    nc.gpsimd.dma_gather(
                        nrg[:].rearrange("p (t q) -> p t q", q=NRW), adl[:],
                        ix_all[:, ixb + (TL + TH) * 8:ixb + IXC],
                        T * 128, T * 128, NRW, single_packet=SP)

                    mff = wpool.tile([128, T], f32, tag="mff")
                    nc.vector.tensor_copy(mff[:], mf_all[:, b * T:(b + 1) * T])
                    S = gpool.tile([128, T * 128], f32, tag="S")
                    nc.vector.tensor_tensor(
                        S[:].rearrange("p (t d) -> p t d", d=128),
                        iof[:].unsqueeze(1).to_broadcast([128, T, 128]),
                        mff[:].unsqueeze(2).to_broadcast([128, T, 128]),
                        Alu.is_equal)

                    n3 = nrg[:].rearrange("p (t q) -> p t q", q=NRW)
                    lg = wpool.tile([128, T * nh], f32, tag="lg")
                    nc.vector.tensor_tensor(
                        lg[:].rearrange("p (t h) -> p t h", h=nh),
                        g3[:, :, nh * c:nh * c + nh], n3[:, :, 0:nh], Alu.add)
                    lg2 = wpool.tile([128, T * nh], f32, tag="lg2")
                    nc.vector.tensor_scalar(lg2[:], lg[:], 0.2, None, Alu.mult)
                    lmax = wpool.tile([128, T * nh], f32, tag="lmax")
                    nc.vector.tensor_tensor(lmax[:], lg[:], lg2[:], Alu.max)

                    pay = gpool.tile([128, T * payw], f32, tag="pay")
                    p3 = pay[:].rearrange("p (t q) -> p t q", q=payw)
                    nc.scalar.activation(
                        p3[:, :, nh * c:nh * c + nh],
                        lmax[:].rearrange("p (t h) -> p t h", h=nh), Act.Exp)
                    nc.vector.tensor_tensor(
                        p3[:, :, 0:nh * c].rearrange("p t (h q) -> p t h q", q=c),
                        g3[:, :, 0:nh * c].rearrange("p t (h q) -> p t h q", q=c),
                        p3[:, :, nh * c:nh * c + nh].unsqueeze(3).to_broadcast(
                            [128, T, nh, c]),
                        Alu.mult)

                    ps = psC.tile([128, payw], f32, tag="agg")
                    for t in range(T):
                        nc.tensor.matmul(
                            ps[:], lhsT=S[:, t * 128:(t + 1) * 128],
                            rhs=pay[:, t * payw:(t + 1) * payw],
                            start=(t == 0), stop=(t == T - 1))

                    den = wpool.tile([128, nh], f32, tag="den")
                    nc.vector.tensor_scalar(den[:], ps[:, nh * c:nh * c + nh],
                                            1e-16, None, Alu.add)
                    rden = wpool.tile([128, nh], f32, tag="rden")
                    nc.vector.reciprocal(rden[:], den[:])
                    orow = wpool.tile([128, outw], f32, tag="orow")
                    nc.vector.tensor_tensor(
                        orow[:].rearrange("p (h q) -> p h q", q=c),
                        ps[:, 0:nh * c].rearrange("p (h q) -> p h q", q=c),
                        rden[:].unsqueeze(2).to_broadcast([128, nh, c]),
                        Alu.mult)
                    ob = wpool.tile([128, outw], f32, tag="ob")
                    nc.vector.tensor_tensor(ob[:], orow[:], bias[:, :outw], Alu.add)
                    ofin = wpool.tile([128, outw], out_dt, tag="ofin")
                    if relu:
                        nc.scalar.activation(ofin[:], ob[:], Act.Relu)
                    else:
                        nc.scalar.activation(ofin[:], ob[:], Act.Copy)
                    nc.gpsimd.indirect_dma_start(
                        out=out_dram[:], out_offset=IOA(ap=lid_all[:, b:b + 1], axis=0),
                        in_=ofin[:], in_offset=None)

            # ---------------- own-shard dense (L2/L3) ------------------------
            def dense_own(x_dram, wlo, whi, outf, own_h, adl, asrc_cols):
                # x_dram [LPAD, F]; own_h [LPAD, fatw]; writes [h|asrc] + adst
                for t in range(NLT):
                    xr = wpool.tile([128, F], f32, tag="xr")
                    nc.sync.dma_start(out=xr[:], in_=x_dram[t * 128:(t + 1) * 128, :])
                    pt0 = psB.tile([128, 128], f32, tag="tr")
                    nc.tensor.transpose(out=pt0[:], in_=xr[:, 0:128], identity=ident[:])
                    xT0 = wpool.tile([128, 128], f32, tag="xT0")
                    nc.scalar.activation(xT0[:], pt0[:], Act.Copy)
                    pt1 = psB.tile([128, 128], f32, tag="tr")
                    nc.tensor.transpose(out=pt1[:], in_=xr[:, 128:256], identity=ident[:])
                    xT1 = wpool.tile([128, 128], f32, tag="xT1")
                    nc.scalar.activation(xT1[:], pt1[:], Act.Copy)
                    nw = wlo.shape[1]
                    ps = psA.tile([128, nw], f32, tag="dens")
                    nc.tensor.matmul(ps[:], lhsT=xT0[:], rhs=wlo[:], start=True, stop=False)
                    nc.tensor.matmul(ps[:], lhsT=xT1[:], rhs=whi[:], start=False, stop=True)
                    nasrc = asrc_cols  # number of asrc cols (nh)
                    hw_ = nw - 2 * nasrc  # h cols
                    fatw_ = own_h.shape[1]
                    hrow = wpool.tile([128, fatw_], f32, tag="hrow2")
                    nc.vector.tensor_copy(hrow[:, :hw_ + nasrc], ps[:, :hw_ + nasrc])
                    nc.vector.memset(hrow[:, hw_ + nasrc:], 0.0)
                    nc.sync.dma_start(out=own_h[t * 128:(t + 1) * 128, :], in_=hrow[:])
                    ad = wpool.tile([128, NRW], f32, tag="ad")
                    nc.scalar.activation(ad[:, 0:nasrc], ps[:, hw_ + nasrc:hw_ + 2 * nasrc], Act.Copy)
                    nc.vector.memset(ad[:, nasrc:], 0.0)
                    nc.sync.dma_start(out=adl[t * 128:(t + 1) * 128, :], in_=ad[:])

            # ================= pipeline =================
            # L1 agg -> own_x1
            nc.sync.dma_start(out=own_x1[SHARD:LPAD, :], in_=zero[:LPAD - SHARD, :])
            agg_layer(tbl1, adl1, FATW, NH, HID, F + NH, b1, True, own_x1, F)

            # L2 dense -> own_h2 (+adl2), fix pad row, allgather -> tbl2
            dense_own(own_x1, w2t_lo, w2t_hi, F, own_h2, adl2, NH)
            nc.sync.dma_start(out=own_h2[SHARD:LPAD, F:F + NH], in_=neg[:LPAD - SHARD, :])
            nc.gpsimd.collective_compute(
                "AllGather", Alu.bypass, replica_groups=[list(range(R))],
                ins=[own_h2[:].opt()], outs=[tbl2[:].opt()])

            # L2 agg -> own_x2
            nc.sync.dma_start(out=own_x2[SHARD:LPAD, :], in_=zero[:LPAD - SHARD, :])
            agg_layer(tbl2, adl2, FATW, NH, HID, F + NH, b2, True, own_x2, F)

            # L3 dense -> own_h3 (+adl3), fix pad row, allgather -> tbl3
            dense_own(own_x2, w3t_lo, w3t_hi, OUT, own_h3, adl3, 1)
            nc.sync.dma_start(out=own_h3[SHARD:LPAD, OUT:OUT + 1], in_=neg[:LPAD - SHARD, 0:1])
            nc.gpsimd.collective_compute(
                "AllGather", Alu.bypass, replica_groups=[list(range(R))],
                ins=[own_h3[:].opt()], outs=[tbl3[:].opt()])

            # L3 agg -> out (bf16 to halve D2H)
            agg_layer(tbl3, adl3, FATW3, 1, OUT, OUT + 1, b3, False, out_p, OUT,
                      out_dt=bf16)

    if not nc.is_finalized():
        nc.finalize()
    return nc


def make_inputs(inputs, cfg):
    """Host prep: returns (nc-ready in_maps list, B)."""
    x = np.asarray(inputs["x"], np.float32)
    edge_index = np.asarray(inputs["edge_index"])
    NH, HID, OUT = cfg["NH"], cfg["HID"], cfg["OUT"]
    per_core, B = prep_host(x, edge_index, cfg)

    w1t = _aug_w(np.asarray(inputs["W1"], np.float32),
                 np.asarray(inputs["as1"], np.float32),
                 np.asarray(inputs["ad1"], np.float32), NH, HID)
    w2t = _aug_w(np.asarray(inputs["W2"], np.float32),
                 np.asarray(inputs["as2"], np.float32),
                 np.asarray(inputs["ad2"], np.float32), NH, HID)
    w3t = _aug_w(np.asarray(inputs["W3"], np.float32),
                 np.asarray(inputs["as3"], np.float32),
                 np.asarray(inputs["ad3"], np.float32), 1, OUT)
    bvec = np.concatenate([np.asarray(inputs["b1"], np.float32),
                           np.asarray(inputs["b2"], np.float32),
                           np.asarray(inputs["b3"], np.float32)])[None, :]

    shared = dict(w1t=w1t, w2t=w2t, w3t=w3t, bvec=bvec)
    in_maps = []
    for r in range(cfg["R"]):
        m = dict(shared)
        m["idxc"] = per_core[r]["idxc"]
        m["mf8"] = per_core[r]["mf8"]
        m["lid32"] = per_core[r]["lid32"]
        m["xmine"] = per_core[r]["xmine"]
        in_maps.append(m)
    return in_maps, B


class Runner:
    """Caches the jitted shard_map executable for a built nc.

    Per call: host-concat per-core inputs, H2D, exec, D2H. Donated output
    buffers are created on device (no H2D cost).
    """

    def __init__(self, nc, n_cores):
        import jax
        import jax.numpy as jnp
        from jax.sharding import Mesh, PartitionSpec, NamedSharding
        from jax.experimental.shard_map import shard_map
        from concourse import bass2jax
        from concourse.bass2jax import _bass_exec_p, install_neuronx_cc_hook

        install_neuronx_cc_hook()
        self.jax = jax
        self.np = np
        self.n_cores = n_cores

        partition_name = (nc.partition_id_tensor.name
                          if nc.partition_id_tensor else None)
        in_names, out_names, out_avals, zero_specs = [], [], [], []
        for alloc in nc.m.functions[0].allocations:
            if not isinstance(alloc, mybir.MemoryLocationSet):
                continue
            name = alloc.memorylocations[0].name
            if alloc.kind == "ExternalInput":
                if name != partition_name:
                    in_names.append(name)
            elif alloc.kind == "ExternalOutput":
                shape = tuple(alloc.tensor_shape)
                dtype = mybir.dt.np(alloc.dtype)
                out_names.append(name)
                out_avals.append(jax.core.ShapedArray(shape, dtype))
                zero_specs.append((shape, dtype))
        n_params = len(in_names)
        n_outs = len(out_avals)
        in_names_all = list(in_names) + list(out_names)
        if partition_name is not None:
            in_names_all.append(partition_name)
        self.in_names = in_names
        self.out_names = out_names
        self.out_avals = out_avals
        self.n_params = n_params

        def _body(*args):
            operands = list(args)
            if partition_name is not None:
                operands.append(bass2jax.partition_id_tensor())
            outs = _bass_exec_p.bind(
                *operands, out_avals=tuple(out_avals),
                in_names=tuple(in_names_all), out_names=tuple(out_names),
                lowering_input_output_aliases=(),
                sim_require_finite=True, sim_require_nnan=True, nc=nc)
            return tuple(outs)

        devices = jax.devices()[:n_cores]
        mesh = Mesh(np.asarray(devices), ("core",))
        pcore = PartitionSpec("core")
        in_specs = (pcore,) * (n_params + n_outs)
        out_specs = (pcore,) * n_outs
        donate = tuple(range(n_params, n_params + n_outs))
        self.sharded = jax.jit(
            shard_map(_body, mesh=mesh, in_specs=in_specs,
                      out_specs=out_specs, check_rep=False),
            donate_argnums=donate, keep_unused=True)

        zshard = tuple(NamedSharding(mesh, pcore) for _ in zero_specs)
        self.zeros_fn = jax.jit(
            lambda: tuple(jnp.zeros((n_cores * s[0], *s[1:]), d)
                          for s, d in zero_specs),
            out_shardings=zshard)

    def run(self, in_maps):
        np_ = self.np
        per_core = [[np_.asarray(m[nm]) for nm in self.in_names]
                    for m in in_maps]
        concat_in = [np_.concatenate([per_core[c][i] for c in range(self.n_cores)],
                                     axis=0)
                     for i in range(self.n_params)]
        zs = self.zeros_fn()
        out_arrs = self.sharded(*concat_in, *zs)
        results = [
            {name: np_.asarray(out_arrs[i]).reshape(
                self.n_cores, *self.out_avals[i].shape)[c]
             for i, name in enumerate(self.out_names)}
            for c in range(self.n_cores)
        ]
        return results


_RUNNER_CACHE = {}


def get_runner(cfg, B):
    key = (cfg["N"], cfg["E"], B)
    if key not in _RUNNER_CACHE:
        nc = build_nc(cfg, B)
        _RUNNER_CACHE[key] = Runner(nc, cfg["R"])
    return _RUNNER_CACHE[key]


def run(inputs, cfg=None):
    cfg = cfg or make_cfg()
    in_maps, B = make_inputs(inputs, cfg)
    runner = get_runner(cfg, B)
    results = runner.run(in_maps)
    outs = [np.asarray(results[r]["out"][:cfg["SHARD"]], np.float32)
            for r in range(cfg["R"])]
    return np.concatenate(outs, axis=0)


def kernel(**inputs):
    return run(inputs).astype(np.float32)


# revision 22
# speedup vs baseline: 1.1673x; 1.1673x over previous
"""3-layer GAT on 8 TRN2 NeuronCores.

Sharding: nodes partitioned by dst across 8 cores; per-destination edge
batches (<=128 distinct dst, fixed tile budget) aggregated via one-hot
selection matmuls in PSUM; per-edge source rows fetched with dma_gather
from a per-layer feature table [h | asrc]; adst fetched from a core-local
narrow table; softmax done unnormalized (exp / segment-sum via an extra
payload column).

Transfer-optimized: the only per-core H2D payloads are the x shard
(transposed), compact 16-partition gather indices (replicated to 128
partitions on device), int8 slot metadata, i32 scatter rows, and the
(small) weights. The full layer-1 feature table is built on device from
an AllGather of the x shards; layers 2/3 AllGather their own h shards.
A jitted shard_map runner is cached so repeat calls skip retracing, and
donated output buffers are created device-side (no H2D for them).
"""
import numpy as np

from concourse import bass, bacc, mybir, tile

f32 = mybir.dt.float32
bf16 = mybir.dt.bfloat16
i8 = mybir.dt.int8
i16 = mybir.dt.int16
i32 = mybir.dt.int32
Alu = mybir.AluOpType
Act = mybir.ActivationFunctionType
IOA = bass.IndirectOffsetOnAxis

FULL_CFG = dict(
    N=50000, IN=128, HID=64, OUT=64, NH=4, E=800000, R=8,
    T=17, TL=11, TH=6,            # tiles per batch: low-src + high-src
    VSPLIT=32768,                 # int16 index reach for the fat gather
)


def make_cfg(**over):
    cfg = dict(FULL_CFG)
    cfg.update(over)
    N, R = cfg["N"], cfg["R"]
    assert N % R == 0
    cfg["SHARD"] = N // R
    # local rows: shard + >=2 pad rows, multiple of 128
    cfg["LPAD"] = ((cfg["SHARD"] + 2 + 127) // 128) * 128
    cfg["TROWS"] = R * cfg["LPAD"]
    cfg["PADROW"] = cfg["SHARD"]  # local pad row (asrc=-1e30 in every shard)
    cfg["TRASH"] = cfg["LPAD"] - 1
    if cfg["TROWS"] <= cfg["VSPLIT"]:
        cfg["VSPLIT"] = cfg["TROWS"]
        cfg["TL"] = cfg["TL"] + cfg["TH"]
        cfg["TH"] = 0
    else:
        assert cfg["TROWS"] - cfg["VSPLIT"] <= 32768
        # need a pad row in the high range: core r covers
        # [r*LPAD, r*LPAD+SHARD) real + pads; find r with pad row >= VSPLIT
        r = 0
        while r * cfg["LPAD"] + cfg["SHARD"] < cfg["VSPLIT"]:
            r += 1
        cfg["PADROW_H"] = r * cfg["LPAD"] + cfg["SHARD"]
        assert cfg["PADROW_H"] >= cfg["VSPLIT"]
    # fat table row widths (fp32, multiple of 64 elems = 256B)
    cfg["FATW12"] = 320   # h(256) | asrc(4) | pad
    cfg["FATW3"] = 128    # h(64) | asrc(1) | pad
    cfg["NRW"] = 64       # narrow adst table row width
    cfg["IXC"] = (cfg["TL"] + cfg["TH"] + cfg["T"]) * 8  # idx cols per batch
    return cfg


def _renum(n, cfg):
    return (n // cfg["SHARD"]) * cfg["LPAD"] + (n % cfg["SHARD"])


def _to_bf16(a):
    import ml_dtypes
    return np.asarray(a, np.float32).astype(ml_dtypes.bfloat16)


def _wrap_idx(idx_flat):
    """dma_gather int16 index layout (compact): ordinal i at [i%16, i//16]."""
    n = len(idx_flat)
    assert n % 16 == 0
    return np.asarray(idx_flat, np.int16).reshape(n // 16, 16).T


def _pack_core(src_g, dst_l, cfg):
    """Pack one core's edges (global renumbered src, local dst) into batches.

    Returns list of dicts with per-batch arrays.
    """
    T, TL, TH = cfg["T"], cfg["TL"], cfg["TH"]
    VS = cfg["VSPLIT"]
    capL, capH = TL * 128, TH * 128
    order = np.argsort(dst_l, kind="stable")
    src_g = src_g[order]
    dst_l = dst_l[order]
    nodes, starts, counts = np.unique(dst_l, return_index=True, return_counts=True)

    batches = []

    def new_batch():
        return dict(nodes=[], eL=[], eH=[], sL=[], sH=[])

    def flush(b):
        if b is None or not b["nodes"]:
            return
        batches.append(b)

    cur = new_batch()
    for node, st, cnt in zip(nodes, starts, counts):
        s = src_g[st:st + cnt]
        low = s < VS
        nL, nH = int(low.sum()), int(cnt - low.sum())
        if (len(cur["nodes"]) >= 128 or len(cur["eL"]) + nL > capL
                or len(cur["eH"]) + nH > capH):
            flush(cur)
            cur = new_batch()
        slot = len(cur["nodes"])
        cur["nodes"].append(int(node))
        cur["eL"].extend(s[low].tolist())
        cur["sL"].extend([slot] * nL)
        cur["eH"].extend(s[~low].tolist())
        cur["sH"].extend([slot] * nH)
    flush(cur)
    return batches


def _batch_arrays(batches, B, cfg):
    """Build stacked per-batch device arrays for one core.

    Returns:
      idxc  [16, B*IXC] i16 — compact gather indices (low | high | dloc per batch)
      mf8   [128, B*T]  i8  — per-edge slot ids
      lid32 [128, B]    i32 — slot -> local out row (TRASH for unused slots)
    """
    T, TL, TH = cfg["T"], cfg["TL"], cfg["TH"]
    VS = cfg["VSPLIT"]
    IXC = cfg["IXC"]
    PAD_L = cfg["PADROW"]          # low-range pad row (core 0 local == global)
    PAD_H = cfg.get("PADROW_H", 0)
    idxc = np.zeros((16, B * IXC), np.int16)
    mf8 = np.zeros((128, B * T), np.int8)
    lid32 = np.full((128, B), cfg["TRASH"], np.int32)
    for bi in range(B):
        if bi < len(batches):
            b = batches[bi]
            nodes = b["nodes"]
            eL, sL = b["eL"], b["sL"]
            eH, sH = b["eH"], b["sH"]
        else:
            nodes, eL, sL, eH, sH = [], [], [], [], []
        # low gather indices (pad with PAD_L)
        iL = np.full(TL * 128, PAD_L, np.int64)
        iL[:len(eL)] = eL
        iH = np.full(TH * 128, max(PAD_H - VS, 0), np.int64)
        if eH:
            iH[:len(eH)] = np.asarray(eH) - VS
        # per-edge slot array in ordinal order (L block then H block)
        slots = np.zeros(T * 128, np.int64)
        slots[:len(sL)] = sL
        slots[TL * 128:TL * 128 + len(sH)] = sH
        # per-edge local-dst row for the narrow adst gather
        nodes_a = np.asarray(nodes, np.int64) if nodes else np.zeros(0, np.int64)
        dloc = np.full(T * 128, PAD_L, np.int64)
        if len(sL):
            dloc[:len(sL)] = nodes_a[np.asarray(sL)]
        if len(sH):
            dloc[TL * 128:TL * 128 + len(sH)] = nodes_a[np.asarray(sH)]
        c = bi * IXC
        idxc[:, c:c + TL * 8] = _wrap_idx(iL); c += TL * 8
        if TH:
            idxc[:, c:c + TH * 8] = _wrap_idx(iH); c += TH * 8
        idxc[:, c:c + T * 8] = _wrap_idx(dloc)
        mf8[:, bi * T:(bi + 1) * T] = slots.reshape(T, 128).T
        lid32[:len(nodes), bi] = nodes
    return idxc, mf8, lid32


def prep_host(x, edge_index, cfg):
    """All host-side sharding prep. Returns (per_core dicts, B)."""
    N, R, SHARD, LPAD = cfg["N"], cfg["R"], cfg["SHARD"], cfg["LPAD"]
    IN = cfg["IN"]
    src = np.concatenate([np.asarray(edge_index[0]), np.arange(N)]).astype(np.int64)
    dst = np.concatenate([np.asarray(edge_index[1]), np.arange(N)]).astype(np.int64)
    src_g = _renum(src, cfg)

    per_core_batches = []
    for r in range(R):
        m = (dst // SHARD) == r
        per_core_batches.append(_pack_core(src_g[m], dst[m] - r * SHARD, cfg))
    B = max(len(b) for b in per_core_batches)

    per_core = []
    for r in range(R):
        idxc, mf8, lid32 = _batch_arrays(per_core_batches[r], B, cfg)
        xm = np.zeros((IN, LPAD), np.float32)
        xm[:, :SHARD] = np.asarray(x[r * SHARD:(r + 1) * SHARD]).T
        per_core.append(dict(idxc=idxc, mf8=mf8, lid32=lid32,
                             xmine=_to_bf16(xm)))
    return per_core, B


def _aug_w(W, a_s, a_d, nh, hid):
    """[inF, outF+2*nh] = [W.T | As | Ad]."""
    inf = W.shape[1]
    Wr = W.reshape(nh, hid, inf)
    As = np.einsum("hci,hc->ih", Wr, a_s)
    Ad = np.einsum("hci,hc->ih", Wr, a_d)
    return np.concatenate([W.T, As, Ad], axis=1).astype(np.float32)


def build_nc(cfg, B):
    N, R = cfg["N"], cfg["R"]
    LPAD, TROWS, SHARD = cfg["LPAD"], cfg["TROWS"], cfg["SHARD"]
    T, TL, TH = cfg["T"], cfg["TL"], cfg["TH"]
    VS = cfg["VSPLIT"]
    NH, HID, OUT, IN = cfg["NH"], cfg["HID"], cfg["OUT"], cfg["IN"]
    F = NH * HID              # 256
    FATW, FATW3, NRW = cfg["FATW12"], cfg["FATW3"], cfg["NRW"]
    IXC = cfg["IXC"]
    NLT = LPAD // 128

    # packed weights: w1t | w2t | w3t | bvec flat f32, sharded across cores
    W1N = IN * (F + 2 * NH)
    W2N = F * (F + 2 * NH)
    W3N = F * (OUT + 2)
    BVN = 2 * F + OUT
    WTOT = W1N + W2N + W3N + BVN
    WPAD = ((WTOT + R - 1) // R) * R
    WSH = WPAD // R

    nc = bacc.Bacc("TRN2", target_bir_lowering=False, debug=False, num_devices=R)

    P = {}
    P["xmine"] = nc.declare_dram_parameter("xmine", [IN, LPAD], bf16, isOutput=False)
    P["wsh"] = nc.declare_dram_parameter("wsh", [1, WSH], f32, isOutput=False)
    P["idxc"] = nc.declare_dram_parameter("idxc", [16, B * IXC], i16, isOutput=False)
    P["mf8"] = nc.declare_dram_parameter("mf8", [128, B * T], i8, isOutput=False)
    P["lid32"] = nc.declare_dram_parameter("lid32", [128, B], i32, isOutput=False)
    out_p = nc.declare_dram_parameter("out", [LPAD, OUT], bf16, isOutput=True)

    x0 = nc.dram_tensor("x0", [IN, LPAD], bf16)
    xg = nc.dram_tensor("xg", [R * IN, LPAD], bf16, addr_space="Shared")
    w0 = nc.dram_tensor("w0", [1, WSH], f32)
    wfull = nc.dram_tensor("wfull", [R, WSH], f32, addr_space="Shared")
    tbl1 = nc.dram_tensor("tbl1", [TROWS, FATW], f32)
    tbl2 = nc.dram_tensor("tbl2", [TROWS, FATW], f32, addr_space="Shared")
    tbl3 = nc.dram_tensor("tbl3", [TROWS, FATW3], f32, addr_space="Shared")
    own_h2 = nc.dram_tensor("own_h2", [LPAD, FATW], f32)
    own_h3 = nc.dram_tensor("own_h3", [LPAD, FATW3], f32)
    adl1 = nc.dram_tensor("adl1", [LPAD, NRW], f32)
    adl2 = nc.dram_tensor("adl2", [LPAD, NRW], f32)
    adl3 = nc.dram_tensor("adl3", [LPAD, NRW], f32)
    own_x1 = nc.dram_tensor("own_x1", [LPAD, F], f32)
    own_x2 = nc.dram_tensor("own_x2", [LPAD, F], f32)

    with tile.TileContext(nc) as tc:
        with tc.tile_pool(name="const", bufs=1) as cpool, \
             tc.tile_pool(name="work", bufs=3) as wpool, \
             tc.tile_pool(name="gath", bufs=2) as gpool, \
             tc.tile_pool(name="psA", bufs=2, space="PSUM") as psA, \
             tc.tile_pool(name="psB", bufs=2, space="PSUM") as psB, \
             tc.tile_pool(name="psC", bufs=2, space="PSUM") as psC:

            # weights arrive sharded; AllGather then unpack from flat layout
            nc.sync.dma_start(out=w0[:], in_=P["wsh"][:])
            nc.gpsimd.collective_compute(
                "AllGather", Alu.bypass, replica_groups=[list(range(R))],
                ins=[w0[:].opt()], outs=[wfull[:].opt()])
            wflat = wfull[:].rearrange("a b -> (a b)")

            def load_w(tag, off, p, q):
                t = cpool.tile([p, q], f32, tag=tag)
                nc.sync.dma_start(
                    out=t[:], in_=wflat[off:off + p * q].rearrange(
                        "(p q) -> p q", q=q))
                return t

            w1t = load_w("w1t", 0, IN, F + 2 * NH)
            w2t_lo = load_w("w2lo", W1N, 128, F + 2 * NH)
            w2t_hi = load_w("w2hi", W1N + 128 * (F + 2 * NH), 128, F + 2 * NH)
            w3t_lo = load_w("w3lo", W1N + W2N, 128, OUT + 2)
            w3t_hi = load_w("w3hi", W1N + W2N + 128 * (OUT + 2), 128, OUT + 2)

            # x shard resident in SBUF (for adl1 build; bf16->f32 cast in DMA)
            xm = cpool.tile([IN, LPAD], f32, tag="xm")
            nc.gpsimd.dma_start(out=xm[:], in_=P["xmine"][:])

            # gather indices: load compact [16, B*IXC] into each 16-partition
            # group (8 replicated loads), resident for all layers
            ix_all = cpool.tile([128, B * IXC], i16, tag="ixall")
            for r8 in range(8):
                nc.sync.dma_start(out=ix_all[r8 * 16:(r8 + 1) * 16, :],
                                  in_=P["idxc"][:])
            mf_all = cpool.tile([128, B * T], i8, tag="mfall")
            nc.sync.dma_start(out=mf_all[:], in_=P["mf8"][:])
            lid_all = cpool.tile([128, B], i32, tag="lidall")
            nc.sync.dma_start(out=lid_all[:], in_=P["lid32"][:])

            # device-built constants: iof (row index ramp), identity, biases
            it_a = cpool.tile([128, 128], i32, tag="ita")
            nc.gpsimd.iota(it_a[:], pattern=[[1, 128]], channel_multiplier=0)
            it_b = cpool.tile([128, 128], i32, tag="itb")
            nc.gpsimd.iota(it_b[:], pattern=[[0, 128]], channel_multiplier=1)
            iof = cpool.tile([128, 128], f32, tag="iof")
            nc.vector.tensor_copy(iof[:], it_a[:])
            pidx = cpool.tile([128, 128], f32, tag="pidx")
            nc.vector.tensor_copy(pidx[:], it_b[:])
            ident = cpool.tile([128, 128], f32, tag="ident")
            nc.vector.tensor_tensor(ident[:], iof[:], pidx[:], Alu.is_equal)

            bv = cpool.tile([1, 2 * F + OUT], f32, tag="bv")
            nc.sync.dma_start(
                out=bv[:], in_=wflat[W1N + W2N + W3N:W1N + W2N + W3N + BVN]
                .rearrange("(p q) -> p q", p=1))
            ones1 = cpool.tile([1, 128], f32, tag="ones1")
            nc.vector.memset(ones1[:], 1.0)
            b1 = cpool.tile([128, F], f32, tag="b1")
            b2 = cpool.tile([128, F], f32, tag="b2")
            b3 = cpool.tile([128, OUT], f32, tag="b3")
            for bias_t, off, w in ((b1, 0, F), (b2, F, F), (b3, 2 * F, OUT)):
                psb = psA.tile([128, F + 2 * NH], f32, tag="dens")
                nc.tensor.matmul(psb[:, :w], lhsT=ones1[:], rhs=bv[:, off:off + w],
                                 start=True, stop=True)
                nc.vector.tensor_copy(bias_t[:], psb[:, :w])

            zero = cpool.tile([128, F], f32, tag="zero")
            nc.vector.memset(zero[:], 0.0)
            neg = cpool.tile([128, NH], f32, tag="neg")
            nc.vector.memset(neg[:], -1e30)

            # -------- AllGather x shards -> xg [R*IN, LPAD] ------------------
            # (collectives cannot read IO tensors; stage through x0)
            nc.sync.dma_start(out=x0[:], in_=P["xmine"][:])
            nc.gpsimd.collective_compute(
                "AllGather", Alu.bypass, replica_groups=[list(range(R))],
                ins=[x0[:].opt()], outs=[xg[:].opt()])

            # ---------------- L1 dense: full table1 = [h1|asrc1] -------------
            for rblk in range(R):
                for t in range(NLT):
                    xc = wpool.tile([IN, 128], f32, tag="xc")
                    nc.gpsimd.dma_start(
                        out=xc[:],
                        in_=xg[rblk * IN:(rblk + 1) * IN, t * 128:(t + 1) * 128])
                    ps = psA.tile([128, F + 2 * NH], f32, tag="dens")
                    nc.tensor.matmul(ps[:], lhsT=xc[:], rhs=w1t[:],
                                     start=True, stop=True)
                    hrow = wpool.tile([128, FATW], f32, tag="hrow")
                    if t % 2 == 0:
                        nc.vector.tensor_copy(hrow[:, :F + NH], ps[:, :F + NH])
                    else:
                        nc.scalar.activation(hrow[:, :F + NH], ps[:, :F + NH], Act.Copy)
                    nc.vector.memset(hrow[:, F + NH:], 0.0)
                    row0 = rblk * LPAD + t * 128
                    nc.sync.dma_start(out=tbl1[row0:row0 + 128, :], in_=hrow[:])
            npad = LPAD - SHARD
            nc.sync.dma_start(out=tbl1[SHARD:LPAD, F:F + NH], in_=neg[:npad, :])
            if TH:
                ph = cfg["PADROW_H"]
                nc.sync.dma_start(out=tbl1[ph:ph + npad, F:F + NH], in_=neg[:npad, :])

            # L1 local adst table (from resident x shard)
            for t in range(NLT):
                ps = psB.tile([128, NH], f32, tag="adl")
                nc.tensor.matmul(ps[:], lhsT=xm[:, t * 128:(t + 1) * 128],
                                 rhs=w1t[:, F + NH:F + 2 * NH],
                                 start=True, stop=True)
                ad = wpool.tile([128, NRW], f32, tag="ad")
                nc.vector.tensor_copy(ad[:, 0:NH], ps[:])
                nc.vector.memset(ad[:, NH:], 0.0)
                nc.sync.dma_start(out=adl1[t * 128:(t + 1) * 128, :], in_=ad[:])

            # ---------------- generic agg layer ------------------------------
            def agg_layer(tbl, adl, fatw, nh, c, payw, bias, relu, out_dram, outw,
                          out_dt=f32):
                # payw = nh*c + nh ; outw = nh*c
                for b in range(B):
                    ixb = b * IXC
                    gat = gpool.tile([128, T * fatw], f32, tag="gat")
                    g3 = gat[:].rearrange("p (t q) -> p t q", q=fatw)
                    SP = False  # single_packet overflows DMA packet limits here
                    nc.gpsimd.dma_gather(
                        g3[:, 0:TL, :], tbl[0:VS, :],
                        ix_all[:, ixb:ixb + TL * 8],
                        TL * 128, TL * 128, fatw, single_packet=SP)
                    if TH:
                        nc.gpsimd.dma_gather(
                            g3[:, TL:T, :], tbl[VS:TROWS, :],
                            ix_all[:, ixb + TL * 8:ixb + (TL + TH) * 8],
                            TH * 128, TH * 128, fatw, single_packet=SP)
                    nrg = gpool.tile([128, T * NRW], f32, tag="nrg")
                    nc.gpsimd.dma_gather(
                        nrg[:].rearrange("p (t q) -> p t q", q=NRW), adl[:],
                        ix_all[:, ixb + (TL + TH) * 8:ixb + IXC],
                        T * 128, T * 128, NRW, single_packet=SP)

                    mff = wpool.tile([128, T], f32, tag="mff")
                    nc.vector.tensor_copy(mff[:], mf_all[:, b * T:(b + 1) * T])
                    S = gpool.tile([128, T * 128], f32, tag="S")
                    nc.vector.tensor_tensor(
                        S[:].rearrange("p (t d) -> p t d", d=128),
                        iof[:].unsqueeze(1).to_broadcast([128, T, 128]),
                        mff[:].unsqueeze(2).to_broadcast([128, T, 128]),
                        Alu.is_equal)

                    n3 = nrg[:].rearrange("p (t q) -> p t q", q=NRW)
                    lg = wpool.tile([128, T * nh], f32, tag="lg")
                    nc.vector.tensor_tensor(
                        lg[:].rearrange("p (t h) -> p t h", h=nh),
                        g3[:, :, nh * c:nh * c + nh], n3[:, :, 0:nh], Alu.add)
                    lg2 = wpool.tile([128, T * nh], f32, tag="lg2")
                    nc.vector.tensor_scalar(lg2[:], lg[:], 0.2, None, Alu.mult)
                    lmax = wpool.tile([128, T * nh], f32, tag="lmax")
                    nc.vector.tensor_tensor(lmax[:], lg[:], lg2[:], Alu.max)

                    pay = gpool.tile([128, T * payw], f32, tag="pay")
                    p3 = pay[:].rearrange("p (t q) -> p t q", q=payw)
                    nc.scalar.activation(
                        p3[:, :, nh * c:nh * c + nh],
                        lmax[:].rearrange("p (t h) -> p t h", h=nh), Act.Exp)
                    nc.vector.tensor_tensor(
                        p3[:, :, 0:nh * c].rearrange("p t (h q) -> p t h q", q=c),
                        g3[:, :, 0:nh * c].rearrange("p t (h q) -> p t h q", q=c),
                        p3[:, :, nh * c:nh * c + nh].unsqueeze(3).to_broadcast(
                            [128, T, nh, c]),
                        Alu.mult)

                    ps = psC.tile([128, payw], f32, tag="agg")
                    for t in range(T):
                        nc.tensor.matmul(
                            ps[:], lhsT=S[:, t * 128:(t + 1) * 128],
                            rhs=pay[:, t * payw:(t + 1) * payw],
                            start=(t == 0), stop=(t == T - 1))

                    den = wpool.tile([128, nh], f32, tag="den")
                    nc.vector.tensor_scalar(den[:], ps[:, nh * c:nh * c + nh],
                                            1e-16, None, Alu.add)
                    rden = wpool.tile([128, nh], f32, tag="rden")
                    nc.vector.reciprocal(rden[:], den[:])
                    orow = wpool.tile([128, outw], f32, tag="orow")
                    nc.vector.tensor_tensor(
                        orow[:].rearrange("p (h q) -> p h q", q=c),
                        ps[:, 0:nh * c].rearrange("p (h q) -> p h q", q=c),
                        rden[:].unsqueeze(2).to_broadcast([128, nh, c]),
                        Alu.mult)
                    ob = wpool.tile([128, outw], f32, tag="ob")
                    nc.vector.tensor_tensor(ob[:], orow[:], bias[:, :outw], Alu.add)
                    ofin = wpool.tile([128, outw], out_dt, tag="ofin")
                    if relu:
                        nc.scalar.activation(ofin[:], ob[:], Act.Relu)
                    else:
                        nc.scalar.activation(ofin[:], ob[:], Act.Copy)
                    nc.gpsimd.indirect_dma_start(
                        out=out_dram[:], out_offset=IOA(ap=lid_all[:, b:b + 1], axis=0),
                        in_=ofin[:], in_offset=None)

            # ---------------- own-shard dense (L2/L3) ------------------------
            def dense_own(x_dram, wlo, whi, outf, own_h, adl, asrc_cols):
                # x_dram [LPAD, F]; own_h [LPAD, fatw]; writes [h|asrc] + adst
                for t in range(NLT):
                    xr = wpool.tile([128, F], f32, tag="xr")
                    nc.sync.dma_start(out=xr[:], in_=x_dram[t * 128:(t + 1) * 128, :])
                    pt0 = psB.tile([128, 128], f32, tag="tr")
                    nc.tensor.transpose(out=pt0[:], in_=xr[:, 0:128], identity=ident[:])
                    xT0 = wpool.tile([128, 128], f32, tag="xT0")
                    nc.scalar.activation(xT0[:], pt0[:], Act.Copy)
                    pt1 = psB.tile([128, 128], f32, tag="tr")
                    nc.tensor.transpose(out=pt1[:], in_=xr[:, 128:256], identity=ident[:])
                    xT1 = wpool.tile([128, 128], f32, tag="xT1")
                    nc.scalar.activation(xT1[:], pt1[:], Act.Copy)
                    nw = wlo.shape[1]
                    ps = psA.tile([128, nw], f32, tag="dens")
                    nc.tensor.matmul(ps[:], lhsT=xT0[:], rhs=wlo[:], start=True, stop=False)
                    nc.tensor.matmul(ps[:], lhsT=xT1[:], rhs=whi[:], start=False, stop=True)
                    nasrc = asrc_cols  # number of asrc cols (nh)
                    hw_ = nw - 2 * nasrc  # h cols
                    fatw_ = own_h.shape[1]
                    hrow = wpool.tile([128, fatw_], f32, tag="hrow2")
                    nc.vector.tensor_copy(hrow[:, :hw_ + nasrc], ps[:, :hw_ + nasrc])
                    nc.vector.memset(hrow[:, hw_ + nasrc:], 0.0)
                    nc.sync.dma_start(out=own_h[t * 128:(t + 1) * 128, :], in_=hrow[:])
                    ad = wpool.tile([128, NRW], f32, tag="ad")
                    nc.scalar.activation(ad[:, 0:nasrc], ps[:, hw_ + nasrc:hw_ + 2 * nasrc], Act.Copy)
                    nc.vector.memset(ad[:, nasrc:], 0.0)
                    nc.sync.dma_start(out=adl[t * 128:(t + 1) * 128, :], in_=ad[:])

            # ================= pipeline =================
            # L1 agg -> own_x1
            nc.sync.dma_start(out=own_x1[SHARD:LPAD, :], in_=zero[:LPAD - SHARD, :])
            agg_layer(tbl1, adl1, FATW, NH, HID, F + NH, b1, True, own_x1, F)

            # L2 dense -> own_h2 (+adl2), fix pad row, allgather -> tbl2
            dense_own(own_x1, w2t_lo, w2t_hi, F, own_h2, adl2, NH)
            nc.sync.dma_start(out=own_h2[SHARD:LPAD, F:F + NH], in_=neg[:LPAD - SHARD, :])
            nc.gpsimd.collective_compute(
                "AllGather", Alu.bypass, replica_groups=[list(range(R))],
                ins=[own_h2[:].opt()], outs=[tbl2[:].opt()])

            # L2 agg -> own_x2
            nc.sync.dma_start(out=own_x2[SHARD:LPAD, :], in_=zero[:LPAD - SHARD, :])
            agg_layer(tbl2, adl2, FATW, NH, HID, F + NH, b2, True, own_x2, F)

            # L3 dense -> own_h3 (+adl3), fix pad row, allgather -> tbl3
            dense_own(own_x2, w3t_lo, w3t_hi, OUT, own_h3, adl3, 1)
            nc.sync.dma_start(out=own_h3[SHARD:LPAD, OUT:OUT + 1], in_=neg[:LPAD - SHARD, 0:1])
            nc.gpsimd.collective_compute(
                "AllGather", Alu.bypass, replica_groups=[list(range(R))],
                ins=[own_h3[:].opt()], outs=[tbl3[:].opt()])

            # L3 agg -> out (bf16 to halve D2H)
            agg_layer(tbl3, adl3, FATW3, 1, OUT, OUT + 1, b3, False, out_p, OUT,
                      out_dt=bf16)

    if not nc.is_finalized():
        nc.finalize()
    return nc


def make_inputs(inputs, cfg):
    """Host prep: returns (nc-ready in_maps list, B)."""
    x = np.asarray(inputs["x"], np.float32)
    edge_index = np.asarray(inputs["edge_index"])
    NH, HID, OUT = cfg["NH"], cfg["HID"], cfg["OUT"]
    per_core, B = prep_host(x, edge_index, cfg)

    w1t = _aug_w(np.asarray(inputs["W1"], np.float32),
                 np.asarray(inputs["as1"], np.float32),
                 np.asarray(inputs["ad1"], np.float32), NH, HID)
    w2t = _aug_w(np.asarray(inputs["W2"], np.float32),
                 np.asarray(inputs["as2"], np.float32),
                 np.asarray(inputs["ad2"], np.float32), NH, HID)
    w3t = _aug_w(np.asarray(inputs["W3"], np.float32),
                 np.asarray(inputs["as3"], np.float32),
                 np.asarray(inputs["ad3"], np.float32), 1, OUT)
    bvec = np.concatenate([np.asarray(inputs["b1"], np.float32),
                           np.asarray(inputs["b2"], np.float32),
                           np.asarray(inputs["b3"], np.float32)])
    R = cfg["R"]
    wflat = np.concatenate([w1t.ravel(), w2t.ravel(), w3t.ravel(), bvec])
    wpad = ((len(wflat) + R - 1) // R) * R
    wflat = np.concatenate([wflat, np.zeros(wpad - len(wflat), np.float32)])
    wshards = wflat.reshape(R, 1, wpad // R)

    in_maps = []
    for r in range(R):
        m = dict(wsh=wshards[r])
        m["idxc"] = per_core[r]["idxc"]
        m["mf8"] = per_core[r]["mf8"]
        m["lid32"] = per_core[r]["lid32"]
        m["xmine"] = per_core[r]["xmine"]
        in_maps.append(m)
    return in_maps, B


class Runner:
    """Caches the jitted shard_map executable for a built nc.

    Per call: host-concat per-core inputs, H2D, exec, D2H. Donated output
    buffers are created on device (no H2D cost).
    """

    def __init__(self, nc, n_cores):
        import jax
        import jax.numpy as jnp
        from jax.sharding import Mesh, PartitionSpec, NamedSharding
        from jax.experimental.shard_map import shard_map
        from concourse import bass2jax
        from concourse.bass2jax import _bass_exec_p, install_neuronx_cc_hook

        install_neuronx_cc_hook()
        self.jax = jax
        self.np = np
        self.n_cores = n_cores

        partition_name = (nc.partition_id_tensor.name
                          if nc.partition_id_tensor else None)
        in_names, out_names, out_avals, zero_specs = [], [], [], []
        for alloc in nc.m.functions[0].allocations:
            if not isinstance(alloc, mybir.MemoryLocationSet):
                continue
            name = alloc.memorylocations[0].name
            if alloc.kind == "ExternalInput":
                if name != partition_name:
                    in_names.append(name)
            elif alloc.kind == "ExternalOutput":
                shape = tuple(alloc.tensor_shape)
                dtype = mybir.dt.np(alloc.dtype)
                out_names.append(name)
                out_avals.append(jax.core.ShapedArray(shape, dtype))
                zero_specs.append((shape, dtype))
        n_params = len(in_names)
        n_outs = len(out_avals)
        in_names_all = list(in_names) + list(out_names)
        if partition_name is not None:
            in_names_all.append(partition_name)
        self.in_names = in_names
        self.out_names = out_names
        self.out_avals = out_avals
        self.n_params = n_params

        def _body(*args):
            operands = list(args)
            if partition_name is not None:
                operands.append(bass2jax.partition_id_tensor())
            outs = _bass_exec_p.bind(
                *operands, out_avals=tuple(out_avals),
                in_names=tuple(in_names_all), out_names=tuple(out_names),
                lowering_input_output_aliases=(),
                sim_require_finite=True, sim_require_nnan=True, nc=nc)
            return tuple(outs)

        devices = jax.devices()[:n_cores]
        mesh = Mesh(np.asarray(devices), ("core",))
        pcore = PartitionSpec("core")
        in_specs = (pcore,) * (n_params + n_outs)
        out_specs = (pcore,) * n_outs
        donate = tuple(range(n_params, n_params + n_outs))
        self.sharded = jax.jit(
            shard_map(_body, mesh=mesh, in_specs=in_specs,
                      out_specs=out_specs, check_rep=False),
            donate_argnums=donate, keep_unused=True)

        zshard = tuple(NamedSharding(mesh, pcore) for _ in zero_specs)
        self.zeros_fn = jax.jit(
            lambda: tuple(jnp.zeros((n_cores * s[0], *s[1:]), d)
                          for s, d in zero_specs),
            out_shardings=zshard)

    def run(self, in_maps):
        np_ = self.np
        per_core = [[np_.asarray(m[nm]) for nm in self.in_names]
                    for m in in_maps]
        concat_in = [np_.concatenate([per_core[c][i] for c in range(self.n_cores)],
                                     axis=0)
                     for i in range(self.n_params)]
        zs = self.zeros_fn()
        out_arrs = self.sharded(*concat_in, *zs)
        # fetch the 8 device shards concurrently (overlaps tunnel RTTs)
        from concurrent.futures import ThreadPoolExecutor
        jobs = []
        for i, name in enumerate(self.out_names):
            d0 = self.out_avals[i].shape[0]
            for sh in out_arrs[i].addressable_shards:
                c = sh.index[0].start // d0 if sh.index and sh.index[0].start else 0
                jobs.append((name, c, sh.data))

        def _get(job):
            name, c, data = job
            return name, c, np_.asarray(data)

        results = [dict() for _ in range(self.n_cores)]
        with ThreadPoolExecutor(max_workers=8) as ex:
            for name, c, arr in ex.map(_get, jobs):
                results[c][name] = arr
        return results


_RUNNER_CACHE = {}


def get_runner(cfg, B):
    key = (cfg["N"], cfg["E"], B)
    if key not in _RUNNER_CACHE:
        nc = build_nc(cfg, B)
        _RUNNER_CACHE[key] = Runner(nc, cfg["R"])
    return _RUNNER_CACHE[key]


def run(inputs, cfg=None):
    cfg = cfg or make_cfg()
    in_maps, B = make_inputs(inputs, cfg)
    runner = get_runner(cfg, B)
    results = runner.run(in_maps)
    outs = [np.asarray(results[r]["out"][:cfg["SHARD"]], np.float32)
            for r in range(cfg["R"])]
    return np.concatenate(outs, axis=0)


def kernel(**inputs):
    return run(inputs).astype(np.float32)


# revision 24
# speedup vs baseline: 1.1821x; 1.0127x over previous
"""3-layer GAT on 8 TRN2 NeuronCores.

Sharding: nodes partitioned by dst across 8 cores; per-destination edge
batches (<=128 distinct dst, fixed tile budget) aggregated via one-hot
selection matmuls in PSUM; per-edge source rows fetched with dma_gather
from a per-layer feature table [h | asrc]; adst fetched from a core-local
narrow table; softmax done unnormalized (exp / segment-sum via an extra
payload column).

Transfer-optimized: the only per-core H2D payloads are the x shard
(transposed), compact 16-partition gather indices (replicated to 128
partitions on device), int8 slot metadata, i32 scatter rows, and the
(small) weights. The full layer-1 feature table is built on device from
an AllGather of the x shards; layers 2/3 AllGather their own h shards.
A jitted shard_map runner is cached so repeat calls skip retracing, and
donated output buffers are created device-side (no H2D for them).
"""
import numpy as np

from concourse import bass, bacc, mybir, tile

f32 = mybir.dt.float32
f16 = mybir.dt.float16
i8 = mybir.dt.int8
i16 = mybir.dt.int16
i32 = mybir.dt.int32
Alu = mybir.AluOpType
Act = mybir.ActivationFunctionType
IOA = bass.IndirectOffsetOnAxis

FULL_CFG = dict(
    N=50000, IN=128, HID=64, OUT=64, NH=4, E=800000, R=8,
    T=17, TL=11, TH=6,            # tiles per batch: low-src + high-src
    VSPLIT=32768,                 # int16 index reach for the fat gather
)


def make_cfg(**over):
    cfg = dict(FULL_CFG)
    cfg.update(over)
    N, R = cfg["N"], cfg["R"]
    assert N % R == 0
    cfg["SHARD"] = N // R
    # local rows: shard + >=2 pad rows, multiple of 128
    cfg["LPAD"] = ((cfg["SHARD"] + 2 + 127) // 128) * 128
    cfg["TROWS"] = R * cfg["LPAD"]
    cfg["PADROW"] = cfg["SHARD"]  # local pad row (asrc=-1e30 in every shard)
    cfg["OROWS"] = cfg["LPAD"]
    cfg["TRASH"] = cfg["OROWS"] - 1
    if cfg["TROWS"] <= cfg["VSPLIT"]:
        cfg["VSPLIT"] = cfg["TROWS"]
        cfg["TL"] = cfg["TL"] + cfg["TH"]
        cfg["TH"] = 0
    else:
        assert cfg["TROWS"] - cfg["VSPLIT"] <= 32768
        # need a pad row in the high range: core r covers
        # [r*LPAD, r*LPAD+SHARD) real + pads; find r with pad row >= VSPLIT
        r = 0
        while r * cfg["LPAD"] + cfg["SHARD"] < cfg["VSPLIT"]:
            r += 1
        cfg["PADROW_H"] = r * cfg["LPAD"] + cfg["SHARD"]
        assert cfg["PADROW_H"] >= cfg["VSPLIT"]
    # fat table row widths (fp32, multiple of 64 elems = 256B)
    cfg["FATW12"] = 320   # h(256) | asrc(4) | pad
    cfg["FATW3"] = 128    # h(64) | asrc(1) | pad
    cfg["NRW"] = 64       # narrow adst table row width
    cfg["IXC"] = (cfg["TL"] + cfg["TH"] + cfg["T"]) * 8  # idx cols per batch
    return cfg


def _renum(n, cfg):
    return (n // cfg["SHARD"]) * cfg["LPAD"] + (n % cfg["SHARD"])


def _to_f16(a):
    return np.asarray(a, np.float32).astype(np.float16)


def _wrap_idx(idx_flat):
    """dma_gather int16 index layout (compact): ordinal i at [i%16, i//16]."""
    n = len(idx_flat)
    assert n % 16 == 0
    return np.asarray(idx_flat, np.int16).reshape(n // 16, 16).T


def _pack_core(src_g, dst_l, cfg):
    """Pack one core's edges (global renumbered src, local dst) into batches.

    Returns list of dicts with per-batch arrays.
    """
    T, TL, TH = cfg["T"], cfg["TL"], cfg["TH"]
    VS = cfg["VSPLIT"]
    capL, capH = TL * 128, TH * 128
    order = np.argsort(dst_l, kind="stable")
    src_g = src_g[order]
    dst_l = dst_l[order]
    nodes, starts, counts = np.unique(dst_l, return_index=True, return_counts=True)

    batches = []

    def new_batch():
        return dict(nodes=[], eL=[], eH=[], sL=[], sH=[])

    def flush(b):
        if b is None or not b["nodes"]:
            return
        batches.append(b)

    cur = new_batch()
    for node, st, cnt in zip(nodes, starts, counts):
        s = src_g[st:st + cnt]
        low = s < VS
        nL, nH = int(low.sum()), int(cnt - low.sum())
        if (len(cur["nodes"]) >= 128 or len(cur["eL"]) + nL > capL
                or len(cur["eH"]) + nH > capH):
            flush(cur)
            cur = new_batch()
        slot = len(cur["nodes"])
        cur["nodes"].append(int(node))
        cur["eL"].extend(s[low].tolist())
        cur["sL"].extend([slot] * nL)
        cur["eH"].extend(s[~low].tolist())
        cur["sH"].extend([slot] * nH)
    flush(cur)
    return batches


def _batch_arrays(batches, B, cfg):
    """Build stacked per-batch device arrays for one core.

    Returns:
      idxc  [16, B*IXC] i16 — compact gather indices (low | high | dloc per batch)
      mf8   [128, B*T]  i8  — per-edge slot ids
      lid32 [128, B]    i32 — slot -> local out row (TRASH for unused slots)
    """
    T, TL, TH = cfg["T"], cfg["TL"], cfg["TH"]
    VS = cfg["VSPLIT"]
    IXC = cfg["IXC"]
    PAD_L = cfg["PADROW"]          # low-range pad row (core 0 local == global)
    PAD_H = cfg.get("PADROW_H", 0)
    idxc = np.zeros((16, B * IXC), np.int16)
    mf8 = np.zeros((128, B * T), np.int8)
    lid32 = np.full((128, B), cfg["TRASH"], np.int32)
    for bi in range(B):
        if bi < len(batches):
            b = batches[bi]
            nodes = b["nodes"]
            eL, sL = b["eL"], b["sL"]
            eH, sH = b["eH"], b["sH"]
        else:
            nodes, eL, sL, eH, sH = [], [], [], [], []
        # low gather indices (pad with PAD_L)
        iL = np.full(TL * 128, PAD_L, np.int64)
        iL[:len(eL)] = eL
        iH = np.full(TH * 128, max(PAD_H - VS, 0), np.int64)
        if eH:
            iH[:len(eH)] = np.asarray(eH) - VS
        # per-edge slot array in ordinal order (L block then H block)
        slots = np.zeros(T * 128, np.int64)
        slots[:len(sL)] = sL
        slots[TL * 128:TL * 128 + len(sH)] = sH
        # per-edge local-dst row for the narrow adst gather
        nodes_a = np.asarray(nodes, np.int64) if nodes else np.zeros(0, np.int64)
        dloc = np.full(T * 128, PAD_L, np.int64)
        if len(sL):
            dloc[:len(sL)] = nodes_a[np.asarray(sL)]
        if len(sH):
            dloc[TL * 128:TL * 128 + len(sH)] = nodes_a[np.asarray(sH)]
        c = bi * IXC
        idxc[:, c:c + TL * 8] = _wrap_idx(iL); c += TL * 8
        if TH:
            idxc[:, c:c + TH * 8] = _wrap_idx(iH); c += TH * 8
        idxc[:, c:c + T * 8] = _wrap_idx(dloc)
        mf8[:, bi * T:(bi + 1) * T] = slots.reshape(T, 128).T
        lid32[:len(nodes), bi] = nodes
    return idxc, mf8, lid32


def prep_host(x, edge_index, cfg):
    """All host-side sharding prep. Returns (per_core dicts, B)."""
    N, R, SHARD, LPAD = cfg["N"], cfg["R"], cfg["SHARD"], cfg["LPAD"]
    IN = cfg["IN"]
    src = np.concatenate([np.asarray(edge_index[0]), np.arange(N)]).astype(np.int64)
    dst = np.concatenate([np.asarray(edge_index[1]), np.arange(N)]).astype(np.int64)
    src_g = _renum(src, cfg)

    per_core_batches = []
    for r in range(R):
        m = (dst // SHARD) == r
        per_core_batches.append(_pack_core(src_g[m], dst[m] - r * SHARD, cfg))
    B = max(len(b) for b in per_core_batches)

    per_core = []
    for r in range(R):
        idxc, mf8, lid32 = _batch_arrays(per_core_batches[r], B, cfg)
        xm = np.zeros((IN, LPAD), np.float32)
        xm[:, :SHARD] = np.asarray(x[r * SHARD:(r + 1) * SHARD]).T
        per_core.append(dict(idxc=idxc, mf8=mf8, lid32=lid32,
                             xmine=_to_f16(xm)))
    return per_core, B


def _aug_w(W, a_s, a_d, nh, hid):
    """[inF, outF+2*nh] = [W.T | As | Ad]."""
    inf = W.shape[1]
    Wr = W.reshape(nh, hid, inf)
    As = np.einsum("hci,hc->ih", Wr, a_s)
    Ad = np.einsum("hci,hc->ih", Wr, a_d)
    return np.concatenate([W.T, As, Ad], axis=1).astype(np.float32)


def build_nc(cfg, B):
    N, R = cfg["N"], cfg["R"]
    LPAD, TROWS, SHARD = cfg["LPAD"], cfg["TROWS"], cfg["SHARD"]
    T, TL, TH = cfg["T"], cfg["TL"], cfg["TH"]
    VS = cfg["VSPLIT"]
    NH, HID, OUT, IN = cfg["NH"], cfg["HID"], cfg["OUT"], cfg["IN"]
    OROWS = cfg["OROWS"]
    F = NH * HID              # 256
    FATW, FATW3, NRW = cfg["FATW12"], cfg["FATW3"], cfg["NRW"]
    IXC = cfg["IXC"]
    NLT = LPAD // 128

    # packed weights: w1t | w2t | w3t | bvec flat f32, sharded across cores
    W1N = IN * (F + 2 * NH)
    W2N = F * (F + 2 * NH)
    W3N = F * (OUT + 2)
    BVN = 2 * F + OUT
    WTOT = W1N + W2N + W3N + BVN
    WPAD = ((WTOT + R - 1) // R) * R
    WSH = WPAD // R

    nc = bacc.Bacc("TRN2", target_bir_lowering=False, debug=False, num_devices=R)

    P = {}
    P["xmine"] = nc.declare_dram_parameter("xmine", [IN, LPAD], f16, isOutput=False)
    P["wsh"] = nc.declare_dram_parameter("wsh", [1, WSH], f32, isOutput=False)
    P["idxc"] = nc.declare_dram_parameter("idxc", [16, B * IXC], i16, isOutput=False)
    P["mf8"] = nc.declare_dram_parameter("mf8", [128, B * T], i8, isOutput=False)
    P["lid32"] = nc.declare_dram_parameter("lid32", [128, B], i32, isOutput=False)
    out_p = nc.declare_dram_parameter("out", [OROWS, OUT], f16, isOutput=True)

    x0 = nc.dram_tensor("x0", [IN, LPAD], f16)
    xg = nc.dram_tensor("xg", [R * IN, LPAD], f16, addr_space="Shared")
    w0 = nc.dram_tensor("w0", [1, WSH], f32)
    wfull = nc.dram_tensor("wfull", [R, WSH], f32, addr_space="Shared")
    tbl1 = nc.dram_tensor("tbl1", [TROWS, FATW], f32)
    tbl2 = nc.dram_tensor("tbl2", [TROWS, FATW], f32, addr_space="Shared")
    tbl3 = nc.dram_tensor("tbl3", [TROWS, FATW3], f32, addr_space="Shared")
    own_h2 = nc.dram_tensor("own_h2", [LPAD, FATW], f32)
    own_h3 = nc.dram_tensor("own_h3", [LPAD, FATW3], f32)
    adl1 = nc.dram_tensor("adl1", [LPAD, NRW], f32)
    adl2 = nc.dram_tensor("adl2", [LPAD, NRW], f32)
    adl3 = nc.dram_tensor("adl3", [LPAD, NRW], f32)
    own_x1 = nc.dram_tensor("own_x1", [LPAD, F], f32)
    own_x2 = nc.dram_tensor("own_x2", [LPAD, F], f32)

    with tile.TileContext(nc) as tc:
        with tc.tile_pool(name="const", bufs=1) as cpool, \
             tc.tile_pool(name="work", bufs=3) as wpool, \
             tc.tile_pool(name="gath", bufs=2) as gpool, \
             tc.tile_pool(name="psA", bufs=2, space="PSUM") as psA, \
             tc.tile_pool(name="psB", bufs=2, space="PSUM") as psB, \
             tc.tile_pool(name="psC", bufs=2, space="PSUM") as psC:

            # weights arrive sharded; AllGather then unpack from flat layout
            nc.sync.dma_start(out=w0[:], in_=P["wsh"][:])
            nc.gpsimd.collective_compute(
                "AllGather", Alu.bypass, replica_groups=[list(range(R))],
                ins=[w0[:].opt()], outs=[wfull[:].opt()])
            wflat = wfull[:].rearrange("a b -> (a b)")

            def load_w(tag, off, p, q):
                t = cpool.tile([p, q], f32, tag=tag)
                nc.sync.dma_start(
                    out=t[:], in_=wflat[off:off + p * q].rearrange(
                        "(p q) -> p q", q=q))
                return t

            w1t = load_w("w1t", 0, IN, F + 2 * NH)
            w2t_lo = load_w("w2lo", W1N, 128, F + 2 * NH)
            w2t_hi = load_w("w2hi", W1N + 128 * (F + 2 * NH), 128, F + 2 * NH)
            w3t_lo = load_w("w3lo", W1N + W2N, 128, OUT + 2)
            w3t_hi = load_w("w3hi", W1N + W2N + 128 * (OUT + 2), 128, OUT + 2)

            # x shard resident in SBUF (for adl1 build; f16->f32 cast in DMA)
            xm = cpool.tile([IN, LPAD], f32, tag="xm")
            nc.gpsimd.dma_start(out=xm[:], in_=P["xmine"][:])

            # gather indices: load compact [16, B*IXC] into each 16-partition
            # group (8 replicated loads), resident for all layers
            ix_all = cpool.tile([128, B * IXC], i16, tag="ixall")
            for r8 in range(8):
                nc.sync.dma_start(out=ix_all[r8 * 16:(r8 + 1) * 16, :],
                                  in_=P["idxc"][:])
            mf_all = cpool.tile([128, B * T], i8, tag="mfall")
            nc.sync.dma_start(out=mf_all[:], in_=P["mf8"][:])
            lid_all = cpool.tile([128, B], i32, tag="lidall")
            nc.sync.dma_start(out=lid_all[:], in_=P["lid32"][:])

            # device-built constants: iof (row index ramp), identity, biases
            it_a = cpool.tile([128, 128], i32, tag="ita")
            nc.gpsimd.iota(it_a[:], pattern=[[1, 128]], channel_multiplier=0)
            it_b = cpool.tile([128, 128], i32, tag="itb")
            nc.gpsimd.iota(it_b[:], pattern=[[0, 128]], channel_multiplier=1)
            iof = cpool.tile([128, 128], f32, tag="iof")
            nc.vector.tensor_copy(iof[:], it_a[:])
            pidx = cpool.tile([128, 128], f32, tag="pidx")
            nc.vector.tensor_copy(pidx[:], it_b[:])
            ident = cpool.tile([128, 128], f32, tag="ident")
            nc.vector.tensor_tensor(ident[:], iof[:], pidx[:], Alu.is_equal)

            bv = cpool.tile([1, 2 * F + OUT], f32, tag="bv")
            nc.sync.dma_start(
                out=bv[:], in_=wflat[W1N + W2N + W3N:W1N + W2N + W3N + BVN]
                .rearrange("(p q) -> p q", p=1))
            ones1 = cpool.tile([1, 128], f32, tag="ones1")
            nc.vector.memset(ones1[:], 1.0)
            b1 = cpool.tile([128, F], f32, tag="b1")
            b2 = cpool.tile([128, F], f32, tag="b2")
            b3 = cpool.tile([128, OUT], f32, tag="b3")
            for bias_t, off, w in ((b1, 0, F), (b2, F, F), (b3, 2 * F, OUT)):
                psb = psA.tile([128, F + 2 * NH], f32, tag="dens")
                nc.tensor.matmul(psb[:, :w], lhsT=ones1[:], rhs=bv[:, off:off + w],
                                 start=True, stop=True)
                nc.vector.tensor_copy(bias_t[:], psb[:, :w])

            zero = cpool.tile([128, F], f32, tag="zero")
            nc.vector.memset(zero[:], 0.0)
            neg = cpool.tile([128, NH], f32, tag="neg")
            nc.vector.memset(neg[:], -1e30)

            # -------- AllGather x shards -> xg [R*IN, LPAD] ------------------
            # (collectives cannot read IO tensors; stage through x0)
            nc.sync.dma_start(out=x0[:], in_=P["xmine"][:])
            nc.gpsimd.collective_compute(
                "AllGather", Alu.bypass, replica_groups=[list(range(R))],
                ins=[x0[:].opt()], outs=[xg[:].opt()])

            # ---------------- L1 dense: full table1 = [h1|asrc1] -------------
            for rblk in range(R):
                for t in range(NLT):
                    xc = wpool.tile([IN, 128], f32, tag="xc")
                    nc.gpsimd.dma_start(
                        out=xc[:],
                        in_=xg[rblk * IN:(rblk + 1) * IN, t * 128:(t + 1) * 128])
                    ps = psA.tile([128, F + 2 * NH], f32, tag="dens")
                    nc.tensor.matmul(ps[:], lhsT=xc[:], rhs=w1t[:],
                                     start=True, stop=True)
                    hrow = wpool.tile([128, FATW], f32, tag="hrow")
                    if t % 2 == 0:
                        nc.vector.tensor_copy(hrow[:, :F + NH], ps[:, :F + NH])
                    else:
                        nc.scalar.activation(hrow[:, :F + NH], ps[:, :F + NH], Act.Copy)
                    nc.vector.memset(hrow[:, F + NH:], 0.0)
                    row0 = rblk * LPAD + t * 128
                    nc.sync.dma_start(out=tbl1[row0:row0 + 128, :], in_=hrow[:])
            npad = LPAD - SHARD
            nc.sync.dma_start(out=tbl1[SHARD:LPAD, F:F + NH], in_=neg[:npad, :])
            if TH:
                ph = cfg["PADROW_H"]
                nc.sync.dma_start(out=tbl1[ph:ph + npad, F:F + NH], in_=neg[:npad, :])

            # L1 local adst table (from resident x shard)
            for t in range(NLT):
                ps = psB.tile([128, NH], f32, tag="adl")
                nc.tensor.matmul(ps[:], lhsT=xm[:, t * 128:(t + 1) * 128],
                                 rhs=w1t[:, F + NH:F + 2 * NH],
                                 start=True, stop=True)
                ad = wpool.tile([128, NRW], f32, tag="ad")
                nc.vector.tensor_copy(ad[:, 0:NH], ps[:])
                nc.vector.memset(ad[:, NH:], 0.0)
                nc.sync.dma_start(out=adl1[t * 128:(t + 1) * 128, :], in_=ad[:])

            # ---------------- generic agg layer ------------------------------
            def agg_layer(tbl, adl, fatw, nh, c, payw, bias, relu, out_dram, outw,
                          out_dt=f32):
                # payw = nh*c + nh ; outw = nh*c
                for b in range(B):
                    ixb = b * IXC
                    gat = gpool.tile([128, T * fatw], f32, tag="gat")
                    g3 = gat[:].rearrange("p (t q) -> p t q", q=fatw)
                    SP = False  # single_packet overflows DMA packet limits here
                    nc.gpsimd.dma_gather(
                        g3[:, 0:TL, :], tbl[0:VS, :],
                        ix_all[:, ixb:ixb + TL * 8],
                        TL * 128, TL * 128, fatw, single_packet=SP)
                    if TH:
                        nc.gpsimd.dma_gather(
                            g3[:, TL:T, :], tbl[VS:TROWS, :],
                            ix_all[:, ixb + TL * 8:ixb + (TL + TH) * 8],
                            TH * 128, TH * 128, fatw, single_packet=SP)
                    nrg = gpool.tile([128, T * NRW], f32, tag="nrg")
                    nc.gpsimd.dma_gather(
                        nrg[:].rearrange("p (t q) -> p t q", q=NRW), adl[:],
                        ix_all[:, ixb + (TL + TH) * 8:ixb + IXC],
                        T * 128, T * 128, NRW, single_packet=SP)

                    mff = wpool.tile([128, T], f32, tag="mff")
                    nc.vector.tensor_copy(mff[:], mf_all[:, b * T:(b + 1) * T])
                    S = gpool.tile([128, T * 128], f32, tag="S")
                    nc.vector.tensor_tensor(
                        S[:].rearrange("p (t d) -> p t d", d=128),
                        iof[:].unsqueeze(1).to_broadcast([128, T, 128]),
                        mff[:].unsqueeze(2).to_broadcast([128, T, 128]),
                        Alu.is_equal)

                    n3 = nrg[:].rearrange("p (t q) -> p t q", q=NRW)
                    lg = wpool.tile([128, T * nh], f32, tag="lg")
                    nc.vector.tensor_tensor(
                        lg[:].rearrange("p (t h) -> p t h", h=nh),
                        g3[:, :, nh * c:nh * c + nh], n3[:, :, 0:nh], Alu.add)
                    lg2 = wpool.tile([128, T * nh], f32, tag="lg2")
                    nc.vector.tensor_scalar(lg2[:], lg[:], 0.2, None, Alu.mult)
                    lmax = wpool.tile([128, T * nh], f32, tag="lmax")
                    nc.vector.tensor_tensor(lmax[:], lg[:], lg2[:], Alu.max)

                    pay = gpool.tile([128, T * payw], f32, tag="pay")
                    p3 = pay[:].rearrange("p (t q) -> p t q", q=payw)
                    nc.scalar.activation(
                        p3[:, :, nh * c:nh * c + nh],
                        lmax[:].rearrange("p (t h) -> p t h", h=nh), Act.Exp)
                    nc.vector.tensor_tensor(
                        p3[:, :, 0:nh * c].rearrange("p t (h q) -> p t h q", q=c),
                        g3[:, :, 0:nh * c].rearrange("p t (h q) -> p t h q", q=c),
                        p3[:, :, nh * c:nh * c + nh].unsqueeze(3).to_broadcast(
                            [128, T, nh, c]),
                        Alu.mult)

                    ps = psC.tile([128, payw], f32, tag="agg")
                    for t in range(T):
                        nc.tensor.matmul(
                            ps[:], lhsT=S[:, t * 128:(t + 1) * 128],
                            rhs=pay[:, t * payw:(t + 1) * payw],
                            start=(t == 0), stop=(t == T - 1))

                    den = wpool.tile([128, nh], f32, tag="den")
                    nc.vector.tensor_scalar(den[:], ps[:, nh * c:nh * c + nh],
                                            1e-16, None, Alu.add)
                    rden = wpool.tile([128, nh], f32, tag="rden")
                    nc.vector.reciprocal(rden[:], den[:])
                    orow = wpool.tile([128, outw], f32, tag="orow")
                    nc.vector.tensor_tensor(
                        orow[:].rearrange("p (h q) -> p h q", q=c),
                        ps[:, 0:nh * c].rearrange("p (h q) -> p h q", q=c),
                        rden[:].unsqueeze(2).to_broadcast([128, nh, c]),
                        Alu.mult)
                    ob = wpool.tile([128, outw], f32, tag="ob")
                    nc.vector.tensor_tensor(ob[:], orow[:], bias[:, :outw], Alu.add)
                    ofin = wpool.tile([128, outw], out_dt, tag="ofin")
                    if relu:
                        nc.scalar.activation(ofin[:], ob[:], Act.Relu)
                    else:
                        nc.scalar.activation(ofin[:], ob[:], Act.Copy)
                    nc.gpsimd.indirect_dma_start(
                        out=out_dram[:], out_offset=IOA(ap=lid_all[:, b:b + 1], axis=0),
                        in_=ofin[:], in_offset=None)

            # ---------------- own-shard dense (L2/L3) ------------------------
            def dense_own(x_dram, wlo, whi, outf, own_h, adl, asrc_cols):
                # x_dram [LPAD, F]; own_h [LPAD, fatw]; writes [h|asrc] + adst
                for t in range(NLT):
                    xr = wpool.tile([128, F], f32, tag="xr")
                    nc.sync.dma_start(out=xr[:], in_=x_dram[t * 128:(t + 1) * 128, :])
                    pt0 = psB.tile([128, 128], f32, tag="tr")
                    nc.tensor.transpose(out=pt0[:], in_=xr[:, 0:128], identity=ident[:])
                    xT0 = wpool.tile([128, 128], f32, tag="xT0")
                    nc.scalar.activation(xT0[:], pt0[:], Act.Copy)
                    pt1 = psB.tile([128, 128], f32, tag="tr")
                    nc.tensor.transpose(out=pt1[:], in_=xr[:, 128:256], identity=ident[:])
                    xT1 = wpool.tile([128, 128], f32, tag="xT1")
                    nc.scalar.activation(xT1[:], pt1[:], Act.Copy)
                    nw = wlo.shape[1]
                    ps = psA.tile([128, nw], f32, tag="dens")
                    nc.tensor.matmul(ps[:], lhsT=xT0[:], rhs=wlo[:], start=True, stop=False)
                    nc.tensor.matmul(ps[:], lhsT=xT1[:], rhs=whi[:], start=False, stop=True)
                    nasrc = asrc_cols  # number of asrc cols (nh)
                    hw_ = nw - 2 * nasrc  # h cols
                    fatw_ = own_h.shape[1]
                    hrow = wpool.tile([128, fatw_], f32, tag="hrow2")
                    nc.vector.tensor_copy(hrow[:, :hw_ + nasrc], ps[:, :hw_ + nasrc])
                    nc.vector.memset(hrow[:, hw_ + nasrc:], 0.0)
                    nc.sync.dma_start(out=own_h[t * 128:(t + 1) * 128, :], in_=hrow[:])
                    ad = wpool.tile([128, NRW], f32, tag="ad")
                    nc.scalar.activation(ad[:, 0:nasrc], ps[:, hw_ + nasrc:hw_ + 2 * nasrc], Act.Copy)
                    nc.vector.memset(ad[:, nasrc:], 0.0)
                    nc.sync.dma_start(out=adl[t * 128:(t + 1) * 128, :], in_=ad[:])

            # ================= pipeline =================
            # L1 agg -> own_x1
            nc.sync.dma_start(out=own_x1[SHARD:LPAD, :], in_=zero[:LPAD - SHARD, :])
            agg_layer(tbl1, adl1, FATW, NH, HID, F + NH, b1, True, own_x1, F)

            # L2 dense -> own_h2 (+adl2), fix pad row, allgather -> tbl2
            dense_own(own_x1, w2t_lo, w2t_hi, F, own_h2, adl2, NH)
            nc.sync.dma_start(out=own_h2[SHARD:LPAD, F:F + NH], in_=neg[:LPAD - SHARD, :])
            nc.gpsimd.collective_compute(
                "AllGather", Alu.bypass, replica_groups=[list(range(R))],
                ins=[own_h2[:].opt()], outs=[tbl2[:].opt()])

            # L2 agg -> own_x2
            nc.sync.dma_start(out=own_x2[SHARD:LPAD, :], in_=zero[:LPAD - SHARD, :])
            agg_layer(tbl2, adl2, FATW, NH, HID, F + NH, b2, True, own_x2, F)

            # L3 dense -> own_h3 (+adl3), fix pad row, allgather -> tbl3
            dense_own(own_x2, w3t_lo, w3t_hi, OUT, own_h3, adl3, 1)
            nc.sync.dma_start(out=own_h3[SHARD:LPAD, OUT:OUT + 1], in_=neg[:LPAD - SHARD, 0:1])
            nc.gpsimd.collective_compute(
                "AllGather", Alu.bypass, replica_groups=[list(range(R))],
                ins=[own_h3[:].opt()], outs=[tbl3[:].opt()])

            # L3 agg -> out (f16 to halve D2H)
            agg_layer(tbl3, adl3, FATW3, 1, OUT, OUT + 1, b3, False, out_p, OUT,
                      out_dt=f16)

    if not nc.is_finalized():
        nc.finalize()
    return nc


def make_inputs(inputs, cfg):
    """Host prep: returns (nc-ready in_maps list, B)."""
    x = np.asarray(inputs["x"], np.float32)
    edge_index = np.asarray(inputs["edge_index"])
    NH, HID, OUT = cfg["NH"], cfg["HID"], cfg["OUT"]
    per_core, B = prep_host(x, edge_index, cfg)

    w1t = _aug_w(np.asarray(inputs["W1"], np.float32),
                 np.asarray(inputs["as1"], np.float32),
                 np.asarray(inputs["ad1"], np.float32), NH, HID)
    w2t = _aug_w(np.asarray(inputs["W2"], np.float32),
                 np.asarray(inputs["as2"], np.float32),
                 np.asarray(inputs["ad2"], np.float32), NH, HID)
    w3t = _aug_w(np.asarray(inputs["W3"], np.float32),
                 np.asarray(inputs["as3"], np.float32),
                 np.asarray(inputs["ad3"], np.float32), 1, OUT)
    bvec = np.concatenate([np.asarray(inputs["b1"], np.float32),
                           np.asarray(inputs["b2"], np.float32),
                           np.asarray(inputs["b3"], np.float32)])
    R = cfg["R"]
    wflat = np.concatenate([w1t.ravel(), w2t.ravel(), w3t.ravel(), bvec])
    wpad = ((len(wflat) + R - 1) // R) * R
    wflat = np.concatenate([wflat, np.zeros(wpad - len(wflat), np.float32)])
    wshards = wflat.reshape(R, 1, wpad // R)

    in_maps = []
    for r in range(R):
        m = dict(wsh=wshards[r])
        m["idxc"] = per_core[r]["idxc"]
        m["mf8"] = per_core[r]["mf8"]
        m["lid32"] = per_core[r]["lid32"]
        m["xmine"] = per_core[r]["xmine"]
        in_maps.append(m)
    return in_maps, B


class Runner:
    """Caches the jitted shard_map executable for a built nc.

    Per call: host-concat per-core inputs, H2D, exec, D2H. Donated output
    buffers are created on device (no H2D cost).
    """

    def __init__(self, nc, n_cores):
        import jax
        import jax.numpy as jnp
        from jax.sharding import Mesh, PartitionSpec, NamedSharding
        from jax.experimental.shard_map import shard_map
        from concourse import bass2jax
        from concourse.bass2jax import _bass_exec_p, install_neuronx_cc_hook

        install_neuronx_cc_hook()
        self.jax = jax
        self.np = np
        self.n_cores = n_cores

        partition_name = (nc.partition_id_tensor.name
                          if nc.partition_id_tensor else None)
        in_names, out_names, out_avals, zero_specs = [], [], [], []
        for alloc in nc.m.functions[0].allocations:
            if not isinstance(alloc, mybir.MemoryLocationSet):
                continue
            name = alloc.memorylocations[0].name
            if alloc.kind == "ExternalInput":
                if name != partition_name:
                    in_names.append(name)
            elif alloc.kind == "ExternalOutput":
                shape = tuple(alloc.tensor_shape)
                dtype = mybir.dt.np(alloc.dtype)
                out_names.append(name)
                out_avals.append(jax.core.ShapedArray(shape, dtype))
                zero_specs.append((shape, dtype))
        n_params = len(in_names)
        n_outs = len(out_avals)
        in_names_all = list(in_names) + list(out_names)
        if partition_name is not None:
            in_names_all.append(partition_name)
        self.in_names = in_names
        self.out_names = out_names
        self.out_avals = out_avals
        self.n_params = n_params

        def _body(*args):
            operands = list(args)
            if partition_name is not None:
                operands.append(bass2jax.partition_id_tensor())
            outs = _bass_exec_p.bind(
                *operands, out_avals=tuple(out_avals),
                in_names=tuple(in_names_all), out_names=tuple(out_names),
                lowering_input_output_aliases=(),
                sim_require_finite=True, sim_require_nnan=True, nc=nc)
            return tuple(outs)

        devices = jax.devices()[:n_cores]
        mesh = Mesh(np.asarray(devices), ("core",))
        pcore = PartitionSpec("core")
        in_specs = (pcore,) * (n_params + n_outs)
        out_specs = (pcore,) * n_outs
        donate = tuple(range(n_params, n_params + n_outs))
        self.sharded = jax.jit(
            shard_map(_body, mesh=mesh, in_specs=in_specs,
                      out_specs=out_specs, check_rep=False),
            donate_argnums=donate, keep_unused=True)

        zshard = tuple(NamedSharding(mesh, pcore) for _ in zero_specs)
        self.zeros_fn = jax.jit(
            lambda: tuple(jnp.zeros((n_cores * s[0], *s[1:]), d)
                          for s, d in zero_specs),
            out_shardings=zshard)

    def run(self, in_maps):
        np_ = self.np
        per_core = [[np_.asarray(m[nm]) for nm in self.in_names]
                    for m in in_maps]
        concat_in = [np_.concatenate([per_core[c][i] for c in range(self.n_cores)],
                                     axis=0)
                     for i in range(self.n_params)]
        zs = self.zeros_fn()
        out_arrs = self.sharded(*concat_in, *zs)
        # fetch the 8 device shards concurrently (overlaps tunnel RTTs)
        from concurrent.futures import ThreadPoolExecutor
        jobs = []
        for i, name in enumerate(self.out_names):
            d0 = self.out_avals[i].shape[0]
            for sh in out_arrs[i].addressable_shards:
                c = sh.index[0].start // d0 if sh.index and sh.index[0].start else 0
                jobs.append((name, c, sh.data))

        def _get(job):
            name, c, data = job
            return name, c, np_.asarray(data)

        results = [dict() for _ in range(self.n_cores)]
        with ThreadPoolExecutor(max_workers=8) as ex:
            for name, c, arr in ex.map(_get, jobs):
                results[c][name] = arr
        return results


_RUNNER_CACHE = {}


def get_runner(cfg, B):
    key = (cfg["N"], cfg["E"], B)
    if key not in _RUNNER_CACHE:
        nc = build_nc(cfg, B)
        _RUNNER_CACHE[key] = Runner(nc, cfg["R"])
    return _RUNNER_CACHE[key]


def run(inputs, cfg=None):
    cfg = cfg or make_cfg()
    in_maps, B = make_inputs(inputs, cfg)
    runner = get_runner(cfg, B)
    results = runner.run(in_maps)
    outs = [np.asarray(results[r]["out"][:cfg["SHARD"]], np.float32)
            for r in range(cfg["R"])]
    return np.concatenate(outs, axis=0)


def kernel(**inputs):
    return run(inputs).astype(np.float32)
